# revision 1
# baseline (speedup 1.0000x reference)
"""Trainium2 Bass kernel for nn_GraphSemanticExtractor (GNN message passing).

Sharding (8 NeuronCores):
  Launch A: edge build        -- core c => (batch b=c//4, row-chunk rc=c%4 of 256 rows)
  Launch B: GAT layer 1       -- core c => (batch b=c//4, head hd=c%4)
  Launch C: GAT layer 2       -- same as B, inputs are B's per-head partial outputs
  Launch D: pool + proj head  -- core c => batch b=c (2 cores)

Key idea: the sparse top-k aggregation out[dst] += wgt*h[src] is done as a dense
matmul out.T = h.T @ R with R[s,t] = ew_k(s)*exp(lrelu(e_src[s]+e_dst[t])) at
t=topi[s,k].  R is built on the vector engine with iota-compare terms
(M0 = sum_k (iota==topi_k)*ew_k) and the attention factor applied densely.
Host-side work between launches is pure gather/transpose/concat glue.
"""

import sys

sys.path.insert(0, "/opt/trn_rl_repo")
sys.path.insert(0, "/opt/trn_rl_repo/concourse")

from contextlib import ExitStack

import ml_dtypes
import numpy as np

import concourse.bass as bass
import concourse.tile as tile
from concourse import bacc, mybir
from concourse.bass_utils import run_bass_kernel_spmd

F32 = mybir.dt.float32
BF16 = mybir.dt.bfloat16
U32 = mybir.dt.uint32
AF = mybir.ActivationFunctionType
OP = mybir.AluOpType
AX = mybir.AxisListType

B, S, H = 2, 1024, 1024
HEADS, K = 4, 8
SEM = 512
NB = H // 128  # 8 partition blocks
CH = S // 4    # 256 rows per edge-build core


def _mm_loop(ctx, nc, psum_pool, lhsT, rhs, mblocks, nsize, kblocks, evict):
    """out[m,n] = sum_k lhsT[k]^T rhs[k].  lhsT(k,m)->AP [128, Mblk], rhs(k,n)->AP [128,nn].
    evict(m, n0, nn, psum_ap) stores the [128, nn] f32 psum tile."""
    for m in range(mblocks):
        n0 = 0
        while n0 < nsize:
            nn = min(512, nsize - n0)
            pt = psum_pool.tile([128, nn], F32, tag="mmp")
            for k in range(kblocks):
                nc.tensor.matmul(
                    pt[:], lhsT(k, m), rhs(k, n0, nn),
                    start=(k == 0), stop=(k == kblocks - 1),
                )
            evict(m, n0, nn, pt[:])
            n0 += nn


def _build_A(nc):
    """Edge build: inputs xT (full, transposed), xTc (row chunk), phi_w.T, psi_w.T."""
    xT = nc.dram_tensor("xT", [H, S], F32, kind="ExternalInput")
    xTc = nc.dram_tensor("xTc", [H, CH], F32, kind="ExternalInput")
    pwT = nc.dram_tensor("pwT", [H, H], F32, kind="ExternalInput")
    swT = nc.dram_tensor("swT", [H, H], F32, kind="ExternalInput")
    srcx = nc.dram_tensor("srcx", [CH, 1], F32, kind="ExternalInput")
    topi = nc.dram_tensor("topi", [CH, K], U32, kind="ExternalOutput")
    ew = nc.dram_tensor("ew", [CH, K], F32, kind="ExternalOutput")

    with tile.TileContext(nc) as tc, ExitStack() as ctx:
        pers = ctx.enter_context(tc.tile_pool(name="pers", bufs=1))
        psum = ctx.enter_context(tc.tile_pool(name="psum", bufs=6, space="PSUM"))

        xT16 = pers.tile([128, NB, S], BF16, tag="xT16")
        xTc16 = pers.tile([128, NB, CH], BF16, tag="xTc16")
        pwT16 = pers.tile([128, NB, H], BF16, tag="pwT16")
        swT16 = pers.tile([128, NB, H], BF16, tag="swT16")
        xTr = xT[:].rearrange("(kb p) s -> p kb s", p=128)
        tmpa = ctx.enter_context(tc.tile_pool(name="tmpa", bufs=3))
        for kb in range(NB):
            stg = tmpa.tile([128, S], F32, tag="stg")
            nc.sync.dma_start(out=stg[:], in_=xTr[:, kb, :])
            nc.vector.tensor_copy(out=xT16[:, kb, :], in_=stg[:])
        nc.gpsimd.dma_start(out=xTc16[:], in_=xTc[:].rearrange("(kb p) s -> p kb s", p=128))
        nc.gpsimd.dma_start(out=pwT16[:], in_=pwT[:].rearrange("(kb p) s -> p kb s", p=128))
        nc.gpsimd.dma_start(out=swT16[:], in_=swT[:].rearrange("(kb p) s -> p kb s", p=128))

        psi16 = pers.tile([128, NB, S], BF16, tag="psi16")   # psi_h.T [e, t]
        phi16 = pers.tile([128, NB, CH], BF16, tag="phi16")  # phi_h.T [e, s-chunk]

        def ev_psi(m, n0, nn, pt):
            eng = nc.scalar if (m + n0) % 2 else nc.vector
            (eng.copy if eng is nc.scalar else eng.tensor_copy)(out=psi16[:, m, n0:n0 + nn], in_=pt)

        _mm_loop(ctx, nc, psum,
                 lambda k, m: swT16[:, k, m * 128:(m + 1) * 128],
                 lambda k, n0, nn: xT16[:, k, n0:n0 + nn],
                 NB, S, NB, ev_psi)

        def ev_phi(m, n0, nn, pt):
            nc.vector.tensor_copy(out=phi16[:, m, n0:n0 + nn], in_=pt)

        _mm_loop(ctx, nc, psum,
                 lambda k, m: pwT16[:, k, m * 128:(m + 1) * 128],
                 lambda k, n0, nn: xTc16[:, k, n0:n0 + nn],
                 NB, CH, NB, ev_phi)

        # scores [s-chunk, t] f32
        sc = pers.tile([128, 2, S], F32, tag="scores")

        def ev_sc(m, n0, nn, pt):
            nc.vector.tensor_copy(out=sc[:, m, n0:n0 + nn], in_=pt)

        _mm_loop(ctx, nc, psum,
                 lambda k, m: phi16[:, k, m * 128:(m + 1) * 128],
                 lambda k, n0, nn: psi16[:, k, n0:n0 + nn],
                 2, S, NB, ev_sc)

        # top-8 per row, softmax over the 8, self-edge mask
        mv = pers.tile([128, 2, K], F32, tag="mv")
        ti = pers.tile([128, 2, K], U32, tag="ti")
        for m in range(2):
            nc.vector.max(mv[:, m, :], sc[:, m, :])
            nc.vector.max_index(ti[:, m, :], mv[:, m, :], sc[:, m, :])
        ex = pers.tile([128, 2, K], F32, tag="ex")
        nc.scalar.activation(ex[:], mv[:], AF.Exp)
        sm = pers.tile([128, 2, 1], F32, tag="sm")
        nc.vector.tensor_reduce(sm[:], ex[:], axis=AX.X, op=OP.add)
        nc.vector.tensor_scalar(sm[:], sm[:], 1e-8, None, op0=OP.add)
        rc = pers.tile([128, 2, 1], F32, tag="rc")
        nc.vector.reciprocal(rc[:], sm[:])
        sx = pers.tile([128, 2, 1], F32, tag="sx")
        nc.sync.dma_start(out=sx[:], in_=srcx[:].rearrange("(m p) c -> p m c", p=128))
        tif = pers.tile([128, 2, K], F32, tag="tif")
        nc.vector.tensor_copy(out=tif[:], in_=ti[:])
        w8 = pers.tile([128, 2, K], F32, tag="w8")
        msk = pers.tile([128, 2, K], F32, tag="msk")
        for m in range(2):
            nc.vector.tensor_scalar(w8[:, m, :], ex[:, m, :], rc[:, m, :], 1e-8, op0=OP.mult, op1=OP.max)
            nc.vector.tensor_scalar(msk[:, m, :], tif[:, m, :], sx[:, m, :], None, op0=OP.is_equal)
            nc.vector.tensor_scalar(msk[:, m, :], msk[:, m, :], -1.0, 1.0, op0=OP.mult, op1=OP.add)
        ewt = pers.tile([128, 2, K], F32, tag="ewt")
        nc.vector.tensor_tensor(ewt[:], w8[:], msk[:], op=OP.mult)
        nc.sync.dma_start(out=topi[:].rearrange("(m p) k -> p m k", p=128), in_=ti[:])
        nc.sync.dma_start(out=ew[:].rearrange("(m p) k -> p m k", p=128), in_=ewt[:])
    nc.compile()
    return nc


def _build_BC(nc, first, skip_r=False, skip_hmm=False, skip_agg=False, skip_dma=False):
    """One GAT layer for one (batch, head).  Outputs gT[feat, node] = (agg/attn)/HEADS, bf16."""
    if first:
        xT = nc.dram_tensor("xT", [H, S], F32, kind="ExternalInput")
    else:
        ps = [nc.dram_tensor(f"p{i}", [H, S], BF16, kind="ExternalInput") for i in range(4)]
    WT = nc.dram_tensor("WT", [H, H], F32, kind="ExternalInput")
    a2r = nc.dram_tensor("a2r", [2, H], F32, kind="ExternalInput")
    tpf = nc.dram_tensor("tpf", [S, K], F32, kind="ExternalInput")
    tpi = nc.dram_tensor("tpi", [S, K], mybir.dt.int16, kind="ExternalInput")
    ewd = nc.dram_tensor("ewd", [S, K], F32, kind="ExternalInput")
    iot = nc.dram_tensor("iot", [1, S], F32, kind="ExternalInput")
    gT = nc.dram_tensor("gT", [H, S], BF16, kind="ExternalOutput")

    with tile.TileContext(nc) as tc, ExitStack() as ctx:
        pers = ctx.enter_context(tc.tile_pool(name="pers", bufs=1))
        tmp = ctx.enter_context(tc.tile_pool(name="tmp", bufs=3))
        psum = ctx.enter_context(tc.tile_pool(name="psum", bufs=5, space="PSUM"))
        psmall = ctx.enter_context(tc.tile_pool(name="psmall", bufs=1, space="PSUM"))

        xT16 = pers.tile([128, NB, S], BF16, tag="xT16")
        if first:
            nc.gpsimd.dma_start(out=xT16[:], in_=xT[:].rearrange("(kb p) s -> p kb s", p=128))
        else:
            for kb in range(NB):
                pin = [tmp.tile([128, S], BF16, tag=f"pin{i}", name=f"pin{i}") for i in range(4)]
                for i in range(4):
                    nc.sync.dma_start(
                        out=pin[i][:],
                        in_=ps[i][:].rearrange("(kb p) s -> p kb s", p=128)[:, kb, :])
                a01 = tmp.tile([128, S], BF16, tag="a01")
                a23 = tmp.tile([128, S], BF16, tag="a23")
                nc.vector.tensor_tensor(a01[:], pin[0][:], pin[1][:], op=OP.add)
                nc.vector.tensor_tensor(a23[:], pin[2][:], pin[3][:], op=OP.add)
                nc.vector.tensor_tensor(a01[:], a01[:], a23[:], op=OP.add)
                nc.scalar.activation(xT16[:, kb, :], a01[:], AF.Relu)

        WT16 = pers.tile([128, NB, H], BF16, tag="WT16")
        nc.gpsimd.dma_start(out=WT16[:], in_=WT[:].rearrange("(kb p) s -> p kb s", p=128))
        a2s = pers.tile([2, H], BF16, tag="a2s")
        nc.gpsimd.dma_start(out=a2s[:], in_=a2r[:])
        asb = pers.tile([128, H], BF16, tag="asb")
        adb = pers.tile([128, H], BF16, tag="adb")
        nc.gpsimd.partition_broadcast(asb[:], a2s[0:1, :])
        a2d1 = pers.tile([1, H], BF16, tag="a2d1")
        nc.sync.dma_start(out=a2d1[:], in_=a2s[1:2, :])
        nc.gpsimd.partition_broadcast(adb[:], a2d1[:])
        tpw = pers.tile([128, NB, K], mybir.dt.int16, tag="tpw")
        nc.sync.dma_start(out=tpw[:], in_=tpi[:].rearrange("(m p) k -> p m k", p=128))
        ews16 = pers.tile([128, NB, K], BF16, tag="ews16")
        nc.gpsimd.dma_start(out=ews16[:], in_=ewd[:].rearrange("(m p) k -> p m k", p=128))

        # h [node, feat] bf16
        h16 = pers.tile([128, NB, H], BF16, tag="h16")

        def ev_h(m, n0, nn, pt):
            eng = (m + n0 // 512) % 2
            if eng:
                nc.scalar.copy(out=h16[:, m, n0:n0 + nn], in_=pt)
            else:
                nc.vector.tensor_copy(out=h16[:, m, n0:n0 + nn], in_=pt)

        if skip_hmm:
            nc.vector.memset(h16[:], 0.0)
        else:
            _mm_loop(ctx, nc, psum,
                     lambda k, m: xT16[:, k, m * 128:(m + 1) * 128],
                     lambda k, n0, nn: WT16[:, k, n0:n0 + nn],
                     NB, H, NB, ev_h)

        # V = W^T [a_src|a_dst] -> [d, 2], via row-wise reductions of WT
        Vf = pers.tile([128, NB, 2], F32, tag="Vf")
        V16 = pers.tile([128, NB, 2], BF16, tag="V16")
        for m in range(NB):
            j1 = tmp.tile([128, H], BF16, tag="j1")
            nc.vector.scalar_tensor_tensor(j1[:], WT16[:, m, :], 1.0, asb[:],
                                           op0=OP.mult, op1=OP.mult,
                                           accum_out=Vf[:, m, 0:1])
            j2 = tmp.tile([128, H], BF16, tag="j2")
            nc.vector.scalar_tensor_tensor(j2[:], WT16[:, m, :], 1.0, adb[:],
                                           op0=OP.mult, op1=OP.mult,
                                           accum_out=Vf[:, m, 1:2])
        nc.vector.tensor_copy(out=V16[:], in_=Vf[:])

        # e_bothT [2, node] = V^T x
        ebT = pers.tile([2, S], F32, tag="ebT")

        def ev_e(m, n0, nn, pt):
            nc.vector.tensor_copy(out=ebT[:, n0:n0 + nn], in_=pt)

        for n0 in range(0, S, 512):
            pt = psmall.tile([2, 512], F32, tag="ebp")
            for k in range(NB):
                nc.tensor.matmul(pt[:], V16[:, k, :], xT16[:, k, n0:n0 + 512],
                                 start=(k == 0), stop=(k == NB - 1))
            ev_e(0, n0, 512, pt[:])

        edst1 = pers.tile([1, S], F32, tag="edst1")
        nc.sync.dma_start(out=edst1[:], in_=ebT[1:2, :])
        edb = pers.tile([128, S], F32, tag="edb")
        nc.gpsimd.partition_broadcast(edb[:], edst1[:])

        ones11 = pers.tile([1, 1], F32, tag="ones11")
        nc.vector.memset(ones11[:], 1.0)
        esc = pers.tile([128, NB, 1], F32, tag="esc")
        for m in range(NB):
            pt = psmall.tile([128, 1], F32, tag="escp")
            nc.tensor.matmul(pt[:], ebT[0:1, m * 128:(m + 1) * 128], ones11[:],
                             start=True, stop=True)
            nc.vector.tensor_copy(out=esc[:, m, :], in_=pt[:])

        # R [s, t] bf16: M0 = sum_k (iota==topi_k)*ew_k, then * exp(lrelu(e_src+e_dst))
        R = pers.tile([128, NB, S], BF16, tag="R")
        for m in range(0 if skip_r else NB):
            m0 = tmp.tile([128, S], BF16, tag="m0")
            nc.gpsimd.local_scatter(m0[:], ews16[:, m, :], tpw[:, m, :],
                                    channels=128, num_elems=S, num_idxs=K)
            zl = tmp.tile([128, S], F32, tag="zl")
            nc.scalar.activation(zl[:], edb[:], AF.Lrelu, bias=esc[:, m, :], alpha=0.2)
            ez = tmp.tile([128, S], BF16, tag="ez")
            nc.scalar.activation(ez[:], zl[:], AF.Exp)
            nc.vector.tensor_tensor(R[:, m, :], m0[:], ez[:], op=OP.mult)

        # attn^T [1, t] = 1^T R ; recip = 0.25 / (attn + 1e-8)
        onesc = pers.tile([128, 1], BF16, tag="onesc")
        nc.vector.memset(onesc[:], 1.0)
        atT = pers.tile([1, S], F32, tag="atT")
        for n0 in range(0, S, 512):
            pt = psmall.tile([1, 512], F32, tag="atp")
            for k in range(NB):
                nc.tensor.matmul(pt[:], onesc[:], R[:, k, n0:n0 + 512],
                                 start=(k == 0), stop=(k == NB - 1))
            nc.vector.tensor_copy(out=atT[:, n0:n0 + 512], in_=pt[:])
        nc.vector.tensor_scalar(atT[:], atT[:], 1e-8, None, op0=OP.add)
        arc = pers.tile([1, S], F32, tag="arc")
        nc.vector.reciprocal(arc[:], atT[:])
        nc.vector.tensor_scalar(arc[:], arc[:], 1.0 / HEADS, None, op0=OP.mult)
        rcb = pers.tile([128, S], F32, tag="rcb")
        nc.gpsimd.partition_broadcast(rcb[:], arc[:])

        # out^T [feat, t] = h^T R, scaled by rcb
        gsb = pers.tile([128, NB, S], BF16, tag="gsb")

        def ev_g(m, n0, nn, pt):
            nc.vector.tensor_tensor(gsb[:, m, n0:n0 + nn], pt, rcb[:, n0:n0 + nn], op=OP.mult)

        if skip_agg:
            nc.vector.memset(gsb[:], 0.0)
        else:
            _mm_loop(ctx, nc, psum,
                     lambda k, m: h16[:, k, m * 128:(m + 1) * 128],
                     lambda k, n0, nn: R[:, k, n0:n0 + nn],
                     NB, S, NB, ev_g)
        nc.sync.dma_start(out=gT[:].rearrange("(m p) t -> p m t", p=128), in_=gsb[:])
    nc.compile()
    return nc


def _build_D(nc):
    """x3 = relu(sum of per-head partials); attention pool over nodes; 2-layer head."""
    from concourse.masks import make_identity
    ps = [nc.dram_tensor(f"p{i}", [H, S], BF16, kind="ExternalInput") for i in range(4)]
    wpc = nc.dram_tensor("wpc", [H, 1], F32, kind="ExternalInput")
    w1T = nc.dram_tensor("w1T", [H, SEM], F32, kind="ExternalInput")
    b1c = nc.dram_tensor("b1c", [SEM, 1], F32, kind="ExternalInput")
    w2T = nc.dram_tensor("w2T", [SEM, SEM], F32, kind="ExternalInput")
    b2c = nc.dram_tensor("b2c", [SEM, 1], F32, kind="ExternalInput")
    res = nc.dram_tensor("res", [SEM, 1], F32, kind="ExternalOutput")

    with tile.TileContext(nc) as tc, ExitStack() as ctx:
        pers = ctx.enter_context(tc.tile_pool(name="pers", bufs=1))
        tmp = ctx.enter_context(tc.tile_pool(name="tmp", bufs=3))
        psum = ctx.enter_context(tc.tile_pool(name="psum", bufs=6, space="PSUM"))

        x3T = pers.tile([128, NB, S], BF16, tag="x3T")
        pt_ = [pers.tile([128, NB, S], BF16, tag=f"pin{i}", name=f"pin{i}") for i in range(4)]
        for i in range(4):
            nc.sync.dma_start(out=pt_[i][:], in_=ps[i][:].rearrange("(kb p) s -> p kb s", p=128))
        for kb in range(NB):
            a01 = tmp.tile([128, S], BF16, tag="a01")
            a23 = tmp.tile([128, S], BF16, tag="a23")
            nc.vector.tensor_tensor(a01[:], pt_[0][:, kb, :], pt_[1][:, kb, :], op=OP.add)
            nc.vector.tensor_tensor(a23[:], pt_[2][:, kb, :], pt_[3][:, kb, :], op=OP.add)
            nc.vector.tensor_tensor(a01[:], a01[:], a23[:], op=OP.add)
            nc.scalar.activation(x3T[:, kb, :], a01[:], AF.Relu)

        wp16 = pers.tile([128, NB, 1], BF16, tag="wp16")
        nc.gpsimd.dma_start(out=wp16[:], in_=wpc[:].rearrange("(kb p) c -> p kb c", p=128))
        psc = pers.tile([1, S], F32, tag="psc")
        for n0 in range(0, S, 512):
            pt = psum.tile([1, 512], F32, tag="sp")
            for k in range(NB):
                nc.tensor.matmul(pt[:], wp16[:, k, :], x3T[:, k, n0:n0 + 512],
                                 start=(k == 0), stop=(k == NB - 1))
            nc.vector.tensor_copy(out=psc[:, n0:n0 + 512], in_=pt[:])

        mx = pers.tile([1, 1], F32, tag="mx")
        nc.vector.tensor_reduce(mx[:], psc[:], axis=AX.X, op=OP.max)
        nmx = pers.tile([1, 1], F32, tag="nmx")
        nc.vector.tensor_scalar(nmx[:], mx[:], -1.0, None, op0=OP.mult)
        ev = pers.tile([1, S], F32, tag="ev")
        nc.scalar.activation(ev[:], psc[:], AF.Exp, bias=nmx[:])
        sm = pers.tile([1, 1], F32, tag="sm")
        nc.vector.tensor_reduce(sm[:], ev[:], axis=AX.X, op=OP.add)
        rc = pers.tile([1, 1], F32, tag="rc")
        nc.vector.reciprocal(rc[:], sm[:])
        alT = pers.tile([1, S], BF16, tag="alT")
        nc.vector.tensor_scalar(alT[:], ev[:], rc[:], None, op0=OP.mult)

        alb = pers.tile([128, S], BF16, tag="alb")
        nc.gpsimd.partition_broadcast(alb[:], alT[:])
        pldf = pers.tile([128, NB, 1], F32, tag="pldf")
        pld = pers.tile([128, NB, 1], BF16, tag="pld")
        for m in range(NB):
            junk = tmp.tile([128, S], BF16, tag="junk")
            nc.vector.scalar_tensor_tensor(junk[:], x3T[:, m, :], 1.0, alb[:],
                                           op0=OP.mult, op1=OP.mult,
                                           accum_out=pldf[:, m, :])
        nc.vector.tensor_copy(out=pld[:], in_=pldf[:])

        w116 = pers.tile([128, NB, SEM], BF16, tag="w116")
        nc.gpsimd.dma_start(out=w116[:], in_=w1T[:].rearrange("(kb p) c -> p kb c", p=128))
        b1f = pers.tile([128, 4, 1], F32, tag="b1f")
        nc.sync.dma_start(out=b1f[:], in_=b1c[:].rearrange("(m p) c -> p m c", p=128))
        hid = pers.tile([128, 4, 1], BF16, tag="hid")
        for m in range(4):
            pt = psum.tile([128, 1], F32, tag="sp")
            for k in range(NB):
                nc.tensor.matmul(pt[:], w116[:, k, m * 128:(m + 1) * 128], pld[:, k, :],
                                 start=(k == 0), stop=(k == NB - 1))
            nc.scalar.activation(hid[:, m, :], pt[:], AF.Relu, bias=b1f[:, m, :])

        w216 = pers.tile([128, 4, SEM], BF16, tag="w216")
        nc.gpsimd.dma_start(out=w216[:], in_=w2T[:].rearrange("(kb p) c -> p kb c", p=128))
        b2f = pers.tile([128, 4, 1], F32, tag="b2f")
        nc.sync.dma_start(out=b2f[:], in_=b2c[:].rearrange("(m p) c -> p m c", p=128))
        rsb = pers.tile([128, 4, 1], F32, tag="rsb")
        for m in range(4):
            pt = psum.tile([128, 1], F32, tag="sp")
            for k in range(4):
                nc.tensor.matmul(pt[:], w216[:, k, m * 128:(m + 1) * 128], hid[:, k, :],
                                 start=(k == 0), stop=(k == 3))
            nc.vector.tensor_tensor(rsb[:, m, :], pt[:], b2f[:, m, :], op=OP.add)
        nc.sync.dma_start(out=res[:].rearrange("(m p) c -> p m c", p=128), in_=rsb[:])
    nc.compile()
    return nc


_PROGS = {}


def _get_progs():
    if not _PROGS:
        def mk():
            return bacc.Bacc("TRN2", target_bir_lowering=False, debug=False,
                             enable_asserts=True, num_devices=8)
        _PROGS["A"] = _build_A(mk())
        _PROGS["B"] = _build_BC(mk(), first=True)
        _PROGS["C"] = _build_BC(mk(), first=False)
        _PROGS["D"] = _build_D(mk())
    return _PROGS


def kernel(hidden_states, phi_w, psi_w, gat_lin_w, gat_att, wp, w1, b1, w2, b2,
           _profile=None):
    f32 = np.float32
    bf16 = ml_dtypes.bfloat16
    hidden_states = np.asarray(hidden_states, f32)
    progs = _get_progs()
    C = lambda a: np.ascontiguousarray(a)
    times = {}

    def run(tag, in_maps, core_ids):
        r = run_bass_kernel_spmd(progs[tag], in_maps, core_ids=core_ids)
        if _profile is not None:
            times[tag] = r.exec_time_ns
        return r.results

    # ---- launch A: edge build ----
    xTb = [C(hidden_states[b].T) for b in range(B)]
    pwT, swT = C(np.asarray(phi_w, f32).T), C(np.asarray(psi_w, f32).T)
    in_a = []
    for c in range(8):
        b, rcn = c // 4, c % 4
        in_a.append({
            "xT": xTb[b], "xTc": C(xTb[b][:, rcn * CH:(rcn + 1) * CH]),
            "pwT": pwT, "swT": swT,
            "srcx": C(np.arange(rcn * CH, (rcn + 1) * CH, dtype=np.float32)[:, None]),
        })
    ra = run("A", in_a, list(range(8)))
    topi = np.stack([np.concatenate([ra[b * 4 + r]["topi"] for r in range(4)], 0) for b in range(B)])
    ew = np.stack([np.concatenate([ra[b * 4 + r]["ew"] for r in range(4)], 0) for b in range(B)])
    topi_f = topi.astype(f32)
    iota = np.arange(S, dtype=f32)[None, :]

    # ---- launches B, C: the two GAT layers ----
    ga = np.asarray(gat_att, f32)
    glw = np.asarray(gat_lin_w, f32)
    prev = None
    for li, tag in enumerate(("B", "C")):
        in_l = []
        for c in range(8):
            b, hd = c // 4, c % 4
            Wm = glw[li, hd * H:(hd + 1) * H, :]
            d = {
                "WT": C(Wm.T),
                "a2r": C(ga[li, hd].reshape(2, H)),
                "tpf": C(topi_f[b]), "tpi": C(topi[b].astype(np.int16)),
                "ewd": C(ew[b]), "iot": C(iota),
            }
            if li == 0:
                d["xT"] = xTb[b]
            else:
                for i in range(4):
                    d[f"p{i}"] = prev[b * 4 + i]
            in_l.append(d)
        rl = run(tag, in_l, list(range(8)))
        prev = [np.asarray(rl[c]["gT"], bf16) for c in range(8)]

    # ---- launch D: pooling + projection head ----
    in_d = []
    for b in range(B):
        d = {f"p{i}": prev[b * 4 + i] for i in range(4)}
        d.update({
            "wpc": C(np.asarray(wp, f32).reshape(H, 1)),
            "w1T": C(np.asarray(w1, f32).T), "b1c": C(np.asarray(b1, f32)[:, None]),
            "w2T": C(np.asarray(w2, f32).T), "b2c": C(np.asarray(b2, f32)[:, None]),
        })
        in_d.append(d)
    rd = run("D", in_d, [0, 1])
    out = np.stack([rd[b]["res"][:, 0].astype(f32) for b in range(B)])
    if _profile is not None:
        _profile.update(times)
    return out



# revision 9
# speedup vs baseline: 1.3061x; 1.3061x over previous
"""Trainium2 Bass kernel for nn_GraphSemanticExtractor (GNN message passing).

Sharding (8 NeuronCores), 5 launches with host-side layout glue between them:
  P0: core c => 128-row chunk of M = phi_w @ psi_w.T, plus V = W^T [a_src|a_dst]
      for (layer l=c//4, head hd=c%4).
  P1 (edge build): core c => (batch b=c//4, 256-row chunk rc=c%4);
      scores = (x_c @ M) @ x.T, top-8, softmax over the 8, self-edge mask.
  P2/P3 (GAT layers 1/2): core c => (batch b=c//4, head hd=c%4); between the
      two, the host computes x1 = relu(sum of per-head partials).
  P4: pool + projection head, core b in {0,1}.

The sparse top-k aggregation out[dst] += wgt*h[src] is a dense matmul
out.T = h.T @ R with R[s,t] = ew_k(s)*exp(lrelu(e_src[s]+e_dst[t])) at
t=topi[s,k]; R is built by scattering ew into M0 (gpsimd local_scatter) and a
dense lrelu/exp of the rank-1 e-grid, all overlapped with the h matmul on PE.
"""

import sys

sys.path.insert(0, "/opt/trn_rl_repo")
sys.path.insert(0, "/opt/trn_rl_repo/concourse")

from contextlib import ExitStack

import ml_dtypes
import numpy as np

import concourse.bass as bass
import concourse.tile as tile
from concourse import bacc, mybir
from concourse.bass_utils import run_bass_kernel_spmd

F32 = mybir.dt.float32
BF16 = mybir.dt.bfloat16
U32 = mybir.dt.uint32
I16 = mybir.dt.int16
AF = mybir.ActivationFunctionType
OP = mybir.AluOpType
AX = mybir.AxisListType

B, S, H = 2, 1024, 1024
HEADS, K = 4, 8
SEM = 512
NB = H // 128  # 8 partition blocks
CH = S // 4    # 256 rows per edge-build core


def _build_P0(nc):
    """Per core: 128 rows of M = phi_w @ psi_w.T, and V = W^T [a_src|a_dst]
    for one (layer, head)."""
    pTc = nc.dram_tensor("pTc", [H, 128], BF16, kind="ExternalInput")
    sT = nc.dram_tensor("sT", [H, H], BF16, kind="ExternalInput")
    Wn = nc.dram_tensor("Wn", [H, H], BF16, kind="ExternalInput")
    a2 = nc.dram_tensor("a2", [H, 2], BF16, kind="ExternalInput")
    Mc = nc.dram_tensor("Mc", [128, H], BF16, kind="ExternalOutput")
    VT = nc.dram_tensor("VT", [2, H], F32, kind="ExternalOutput")

    with tile.TileContext(nc) as tc, ExitStack() as ctx:
        pers = ctx.enter_context(tc.tile_pool(name="pers", bufs=1))
        psum = ctx.enter_context(tc.tile_pool(name="psum", bufs=4, space="PSUM"))

        a2t = pers.tile([128, NB, 2], BF16, tag="a2t")
        nc.gpsimd.dma_start(out=a2t[:], in_=a2[:].rearrange("(kb p) c -> p kb c", p=128))
        pT16 = pers.tile([128, NB, 128], BF16, tag="pT16")
        nc.gpsimd.dma_start(out=pT16[:], in_=pTc[:].rearrange("(kb p) c -> p kb c", p=128))
        sT16 = pers.tile([128, NB, H], BF16, tag="sT16")
        nc.sync.dma_start(out=sT16[:], in_=sT[:].rearrange("(kb p) s -> p kb s", p=128))
        Wn16 = pers.tile([128, NB, H], BF16, tag="Wn16")
        nc.sync.dma_start(out=Wn16[:], in_=Wn[:].rearrange("(kb p) s -> p kb s", p=128))

        Mc16 = pers.tile([128, H], BF16, tag="Mc16")
        for n0 in range(0, H, 512):
            pt = psum.tile([128, 512], F32, tag="mm")
            for k in range(NB):
                nc.tensor.matmul(pt[:], pT16[:, k, :], sT16[:, k, n0:n0 + 512],
                                 start=(k == 0), stop=(k == NB - 1))
            nc.vector.tensor_copy(out=Mc16[:, n0:n0 + 512], in_=pt[:])
        nc.sync.dma_start(out=Mc[:], in_=Mc16[:])

        Vt = pers.tile([2, H], F32, tag="Vt")
        for n0 in range(0, H, 512):
            pt = psum.tile([2, 512], F32, tag="vm")
            for k in range(NB):
                nc.tensor.matmul(pt[:], a2t[:, k, :], Wn16[:, k, n0:n0 + 512],
                                 start=(k == 0), stop=(k == NB - 1))
            nc.vector.tensor_copy(out=Vt[:, n0:n0 + 512], in_=pt[:])
        nc.sync.dma_start(out=VT[:], in_=Vt[:])
    nc.compile()
    return nc


def _build_P1(nc):
    """Edge build: scores = (x_c @ M) @ x.T, top-8 + softmax + self-mask."""
    xT = nc.dram_tensor("xT", [H, S], BF16, kind="ExternalInput")
    xTc = nc.dram_tensor("xTc", [H, CH], BF16, kind="ExternalInput")
    Mm = nc.dram_tensor("Mm", [H, H], BF16, kind="ExternalInput")
    srcx = nc.dram_tensor("srcx", [CH, 1], F32, kind="ExternalInput")
    topi = nc.dram_tensor("topi", [CH, K], U32, kind="ExternalOutput")
    ew = nc.dram_tensor("ew", [CH, K], F32, kind="ExternalOutput")

    with tile.TileContext(nc) as tc, ExitStack() as ctx:
        pers = ctx.enter_context(tc.tile_pool(name="pers", bufs=1))
        psum = ctx.enter_context(tc.tile_pool(name="psum", bufs=4, space="PSUM"))
        psumb = ctx.enter_context(tc.tile_pool(name="psumb", bufs=4, space="PSUM"))

        sx = pers.tile([128, 2, 1], F32, tag="sx")
        nc.gpsimd.dma_start(out=sx[:], in_=srcx[:].rearrange("(m p) c -> p m c", p=128))
        xTc16 = pers.tile([128, NB, CH], BF16, tag="xTc16")
        nc.gpsimd.dma_start(out=xTc16[:], in_=xTc[:].rearrange("(kb p) s -> p kb s", p=128))
        M16 = pers.tile([128, NB, H], BF16, tag="M16")
        nc.sync.dma_start(out=M16[:], in_=Mm[:].rearrange("(kb p) s -> p kb s", p=128))
        xT16 = pers.tile([128, NB, S], BF16, tag="xT16")
        nc.sync.dma_start(out=xT16[:], in_=xT[:].rearrange("(kb p) s -> p kb s", p=128))

        # PT[j, s-chunk] = (x_c @ M).T  (j = feature of M's column space)
        PT16 = pers.tile([128, NB, CH], BF16, tag="PT16")
        for m in range(NB):
            pt = psumb.tile([128, CH], F32, tag="ptm")
            for k in range(NB):
                nc.tensor.matmul(pt[:], M16[:, k, m * 128:(m + 1) * 128], xTc16[:, k, :],
                                 start=(k == 0), stop=(k == NB - 1))
            nc.vector.tensor_copy(out=PT16[:, m, :], in_=pt[:])

        # scores [s-chunk, t] f32
        sc = pers.tile([128, 2, S], F32, tag="scores")
        for sb in range(2):
            for n0 in range(0, S, 512):
                pt = psum.tile([128, 512], F32, tag="scm")
                for k in range(NB):
                    nc.tensor.matmul(pt[:], PT16[:, k, sb * 128:(sb + 1) * 128],
                                     xT16[:, k, n0:n0 + 512],
                                     start=(k == 0), stop=(k == NB - 1))
                nc.vector.tensor_copy(out=sc[:, sb, n0:n0 + 512], in_=pt[:])

        # top-8 per row, softmax over the 8, self-edge mask
        mv = pers.tile([128, 2, K], F32, tag="mv")
        ti = pers.tile([128, 2, K], U32, tag="ti")
        for m in range(2):
            nc.vector.max(mv[:, m, :], sc[:, m, :])
            nc.vector.max_index(ti[:, m, :], mv[:, m, :], sc[:, m, :])
        ex = pers.tile([128, 2, K], F32, tag="ex")
        nc.scalar.activation(ex[:], mv[:], AF.Exp)
        sm = pers.tile([128, 2, 1], F32, tag="sm")
        nc.vector.tensor_reduce(sm[:], ex[:], axis=AX.X, op=OP.add)
        nc.vector.tensor_scalar(sm[:], sm[:], 1e-8, None, op0=OP.add)
        rc = pers.tile([128, 2, 1], F32, tag="rc")
        nc.vector.reciprocal(rc[:], sm[:])
        tif = pers.tile([128, 2, K], F32, tag="tif")
        nc.vector.tensor_copy(out=tif[:], in_=ti[:])
        w8 = pers.tile([128, 2, K], F32, tag="w8")
        msk = pers.tile([128, 2, K], F32, tag="msk")
        for m in range(2):
            nc.vector.tensor_scalar(w8[:, m, :], ex[:, m, :], rc[:, m, :], 1e-8, op0=OP.mult, op1=OP.max)
            nc.vector.tensor_scalar(msk[:, m, :], tif[:, m, :], sx[:, m, :], None, op0=OP.is_equal)
            nc.vector.tensor_scalar(msk[:, m, :], msk[:, m, :], -1.0, 1.0, op0=OP.mult, op1=OP.add)
        ewt = pers.tile([128, 2, K], F32, tag="ewt")
        nc.vector.tensor_tensor(ewt[:], w8[:], msk[:], op=OP.mult)
        nc.sync.dma_start(out=topi[:].rearrange("(m p) k -> p m k", p=128), in_=ti[:])
        nc.sync.dma_start(out=ew[:].rearrange("(m p) k -> p m k", p=128), in_=ewt[:])
    nc.compile()
    return nc


def _build_L(nc):
    """One GAT layer for one (batch, head).  gT[feat, node] = (agg/attn)/HEADS."""
    xT = nc.dram_tensor("xT", [H, S], BF16, kind="ExternalInput")
    WT = nc.dram_tensor("WT", [H, H], BF16, kind="ExternalInput")
    V2 = nc.dram_tensor("V2", [H, 2], BF16, kind="ExternalInput")
    tpi = nc.dram_tensor("tpi", [S, K], I16, kind="ExternalInput")
    ewd = nc.dram_tensor("ewd", [S, K], BF16, kind="ExternalInput")
    gT = nc.dram_tensor("gT", [H, S], BF16, kind="ExternalOutput")

    with tile.TileContext(nc) as tc, ExitStack() as ctx:
        pers = ctx.enter_context(tc.tile_pool(name="pers", bufs=1))
        psum = ctx.enter_context(tc.tile_pool(name="psum", bufs=4, space="PSUM"))
        psmall = ctx.enter_context(tc.tile_pool(name="psmall", bufs=1, space="PSUM"))

        # tiny inputs first on the gpsimd queue, big ones on sync
        tpw = pers.tile([128, NB, K], I16, tag="tpw")
        nc.gpsimd.dma_start(out=tpw[:], in_=tpi[:].rearrange("(m p) k -> p m k", p=128))
        ews16 = pers.tile([128, NB, K], BF16, tag="ews16")
        nc.gpsimd.dma_start(out=ews16[:], in_=ewd[:].rearrange("(m p) k -> p m k", p=128))
        V16 = pers.tile([128, NB, 2], BF16, tag="V16")
        nc.gpsimd.dma_start(out=V16[:], in_=V2[:].rearrange("(kb p) c -> p kb c", p=128))
        xT16 = pers.tile([128, NB, S], BF16, tag="xT16")
        nc.sync.dma_start(out=xT16[:], in_=xT[:].rearrange("(kb p) s -> p kb s", p=128))
        WT16 = pers.tile([128, NB, H], BF16, tag="WT16")
        nc.sync.dma_start(out=WT16[:], in_=WT[:].rearrange("(kb p) s -> p kb s", p=128))

        # gpsimd: M0 blocks (scatter of ew into dense [s, t])
        M0 = pers.tile([128, NB, S], BF16, tag="M0")
        for m in range(NB):
            nc.gpsimd.local_scatter(M0[:, m, :], ews16[:, m, :], tpw[:, m, :],
                                    channels=128, num_elems=S, num_idxs=K)

        # PE: e_srcT / e_dstT [1, node] = (V col)^T x  (separate 1-row outputs so
        # each lands at base partition 0)
        esT = pers.tile([1, S], F32, tag="esT")
        edT = pers.tile([1, S], F32, tag="edT")
        for col, dst in ((1, edT), (0, esT)):
            for n0 in range(0, S, 512):
                pt = psmall.tile([1, 512], F32, tag="ebp")
                for k in range(NB):
                    nc.tensor.matmul(pt[:], V16[:, k, col:col + 1], xT16[:, k, n0:n0 + 512],
                                     start=(k == 0), stop=(k == NB - 1))
                nc.vector.tensor_copy(out=dst[:, n0:n0 + 512], in_=pt[:])

        # PE: broadcast e_dst across partitions (rank-1 matmul with ones)
        ones1r = pers.tile([1, 128], F32, tag="ones1r")
        nc.vector.memset(ones1r[:], 1.0)
        edb = pers.tile([128, S], F32, tag="edb")
        for n0 in range(0, S, 512):
            pt = psmall.tile([128, 512], F32, tag="edbp")
            nc.tensor.matmul(pt[:], ones1r[:], edT[0:1, n0:n0 + 512], start=True, stop=True)
            nc.scalar.copy(out=edb[:, n0:n0 + 512], in_=pt[:])

        # PE: e_src into partitions (transpose via 1-col matmul)
        ones11 = pers.tile([1, 1], F32, tag="ones11")
        nc.vector.memset(ones11[:], 1.0)
        esc = pers.tile([128, NB, 1], F32, tag="esc")
        for m in range(NB):
            pt = psmall.tile([128, 1], F32, tag="escp")
            nc.tensor.matmul(pt[:], esT[0:1, m * 128:(m + 1) * 128], ones11[:],
                             start=True, stop=True)
            nc.vector.tensor_copy(out=esc[:, m, :], in_=pt[:])

        # PE: h [node, feat] bf16 (evictions on DVE only, Act stays free for R)
        h16 = pers.tile([128, NB, H], BF16, tag="h16")
        for m in range(NB):
            for n0 in range(0, H, 512):
                pt = psum.tile([128, 512], F32, tag="mmp")
                for k in range(NB):
                    nc.tensor.matmul(pt[:], xT16[:, k, m * 128:(m + 1) * 128],
                                     WT16[:, k, n0:n0 + 512],
                                     start=(k == 0), stop=(k == NB - 1))
                nc.vector.tensor_copy(out=h16[:, m, n0:n0 + 512], in_=pt[:])

        # Act: all lrelu then all exp (2 act-table loads total); DVE: R = M0 * ez
        zl8 = pers.tile([128, NB, S], F32, tag="zl8")
        for m in range(NB):
            nc.scalar.activation(zl8[:, m, :], edb[:], AF.Lrelu, bias=esc[:, m, :], alpha=0.2)
        ez8 = pers.tile([128, NB, S], BF16, tag="ez8")
        for m in range(NB):
            nc.scalar.activation(ez8[:, m, :], zl8[:, m, :], AF.Exp)
        R = pers.tile([128, NB, S], BF16, tag="R")
        for m in range(NB):
            nc.vector.tensor_tensor(R[:, m, :], M0[:, m, :], ez8[:, m, :], op=OP.mult)

        # PE: attn^T [1, t] = 1^T R ; arc = 0.25 / (attn + 1e-8)
        onesc = pers.tile([128, 1], BF16, tag="onesc")
        nc.vector.memset(onesc[:], 1.0)
        atT = pers.tile([1, S], F32, tag="atT")
        for n0 in range(0, S, 512):
            pt = psmall.tile([1, 512], F32, tag="atp")
            for k in range(NB):
                nc.tensor.matmul(pt[:], onesc[:], R[:, k, n0:n0 + 512],
                                 start=(k == 0), stop=(k == NB - 1))
            nc.vector.tensor_copy(out=atT[:, n0:n0 + 512], in_=pt[:])
        nc.vector.tensor_scalar(atT[:], atT[:], 1e-8, None, op0=OP.add)
        arc = pers.tile([1, S], F32, tag="arc")
        nc.vector.reciprocal(arc[:], atT[:])
        nc.vector.tensor_scalar(arc[:], arc[:], 1.0 / HEADS, None, op0=OP.mult)
        rcb = pers.tile([128, S], F32, tag="rcb")
        nc.gpsimd.partition_broadcast(rcb[:], arc[:])

        # PE: out^T [feat, t] = h^T R, scaled by rcb; chunked DMA out
        gsb = pers.tile([128, NB, S], BF16, tag="gsb")
        gTr = gT[:].rearrange("(m p) t -> p m t", p=128)
        for m in range(NB):
            for n0 in range(0, S, 512):
                pt = psum.tile([128, 512], F32, tag="mmp")
                for k in range(NB):
                    nc.tensor.matmul(pt[:], h16[:, k, m * 128:(m + 1) * 128],
                                     R[:, k, n0:n0 + 512],
                                     start=(k == 0), stop=(k == NB - 1))
                nc.vector.tensor_tensor(gsb[:, m, n0:n0 + 512], pt[:], rcb[:, n0:n0 + 512], op=OP.mult)
            nc.sync.dma_start(out=gTr[:, m, :], in_=gsb[:, m, :])
    nc.compile()
    return nc


def _build_D(nc):
    """Attention pool over nodes + 2-layer projection head, one batch per core."""
    x2T = nc.dram_tensor("x2T", [H, S], BF16, kind="ExternalInput")
    wpc = nc.dram_tensor("wpc", [H, 1], BF16, kind="ExternalInput")
    w1T = nc.dram_tensor("w1T", [H, SEM], BF16, kind="ExternalInput")
    b1c = nc.dram_tensor("b1c", [SEM, 1], F32, kind="ExternalInput")
    w2T = nc.dram_tensor("w2T", [SEM, SEM], BF16, kind="ExternalInput")
    b2c = nc.dram_tensor("b2c", [SEM, 1], F32, kind="ExternalInput")
    res = nc.dram_tensor("res", [SEM, 1], F32, kind="ExternalOutput")

    with tile.TileContext(nc) as tc, ExitStack() as ctx:
        pers = ctx.enter_context(tc.tile_pool(name="pers", bufs=1))
        tmp = ctx.enter_context(tc.tile_pool(name="tmp", bufs=3))
        psum = ctx.enter_context(tc.tile_pool(name="psum", bufs=3, space="PSUM"))

        wp16 = pers.tile([128, NB, 1], BF16, tag="wp16")
        nc.gpsimd.dma_start(out=wp16[:], in_=wpc[:].rearrange("(kb p) c -> p kb c", p=128))
        b1f = pers.tile([128, 4, 1], F32, tag="b1f")
        nc.gpsimd.dma_start(out=b1f[:], in_=b1c[:].rearrange("(m p) c -> p m c", p=128))
        b2f = pers.tile([128, 4, 1], F32, tag="b2f")
        nc.gpsimd.dma_start(out=b2f[:], in_=b2c[:].rearrange("(m p) c -> p m c", p=128))
        x3T = pers.tile([128, NB, S], BF16, tag="x3T")
        nc.sync.dma_start(out=x3T[:], in_=x2T[:].rearrange("(kb p) s -> p kb s", p=128))
        w116 = pers.tile([128, NB, SEM], BF16, tag="w116")
        nc.sync.dma_start(out=w116[:], in_=w1T[:].rearrange("(kb p) c -> p kb c", p=128))
        w216 = pers.tile([128, 4, SEM], BF16, tag="w216")
        nc.sync.dma_start(out=w216[:], in_=w2T[:].rearrange("(kb p) c -> p kb c", p=128))

        psc = pers.tile([1, S], F32, tag="psc")
        for n0 in range(0, S, 512):
            pt = psum.tile([1, 512], F32, tag="sp")
            for k in range(NB):
                nc.tensor.matmul(pt[:], wp16[:, k, :], x3T[:, k, n0:n0 + 512],
                                 start=(k == 0), stop=(k == NB - 1))
            nc.vector.tensor_copy(out=psc[:, n0:n0 + 512], in_=pt[:])

        mx = pers.tile([1, 1], F32, tag="mx")
        nc.vector.tensor_reduce(mx[:], psc[:], axis=AX.X, op=OP.max)
        nmx = pers.tile([1, 1], F32, tag="nmx")
        nc.vector.tensor_scalar(nmx[:], mx[:], -1.0, None, op0=OP.mult)
        ev = pers.tile([1, S], F32, tag="ev")
        nc.scalar.activation(ev[:], psc[:], AF.Exp, bias=nmx[:])
        sm = pers.tile([1, 1], F32, tag="sm")
        nc.vector.tensor_reduce(sm[:], ev[:], axis=AX.X, op=OP.add)
        rc = pers.tile([1, 1], F32, tag="rc")
        nc.vector.reciprocal(rc[:], sm[:])
        alT = pers.tile([1, S], BF16, tag="alT")
        nc.vector.tensor_scalar(alT[:], ev[:], rc[:], None, op0=OP.mult)

        alb = pers.tile([128, S], BF16, tag="alb")
        nc.gpsimd.partition_broadcast(alb[:], alT[:])
        pldf = pers.tile([128, NB, 1], F32, tag="pldf")
        pld = pers.tile([128, NB, 1], BF16, tag="pld")
        for m in range(NB):
            junk = tmp.tile([128, S], BF16, tag="junk")
            nc.vector.scalar_tensor_tensor(junk[:], x3T[:, m, :], 1.0, alb[:],
                                           op0=OP.mult, op1=OP.mult,
                                           accum_out=pldf[:, m, :])
        nc.vector.tensor_copy(out=pld[:], in_=pldf[:])

        hid = pers.tile([128, 4, 1], BF16, tag="hid")
        for m in range(4):
            pt = psum.tile([128, 1], F32, tag="sp1")
            for k in range(NB):
                nc.tensor.matmul(pt[:], w116[:, k, m * 128:(m + 1) * 128], pld[:, k, :],
                                 start=(k == 0), stop=(k == NB - 1))
            nc.scalar.activation(hid[:, m, :], pt[:], AF.Relu, bias=b1f[:, m, :])

        rsb = pers.tile([128, 4, 1], F32, tag="rsb")
        for m in range(4):
            pt = psum.tile([128, 1], F32, tag="sp1")
            for k in range(4):
                nc.tensor.matmul(pt[:], w216[:, k, m * 128:(m + 1) * 128], hid[:, k, :],
                                 start=(k == 0), stop=(k == 3))
            nc.vector.tensor_tensor(rsb[:, m, :], pt[:], b2f[:, m, :], op=OP.add)
        nc.sync.dma_start(out=res[:].rearrange("(m p) c -> p m c", p=128), in_=rsb[:])
    nc.compile()
    return nc


_PROGS = {}


def _get_progs():
    if not _PROGS:
        def mk():
            return bacc.Bacc("TRN2", target_bir_lowering=False, debug=False,
                             enable_asserts=True, num_devices=8)
        _PROGS["A0"] = _build_P0(mk())
        _PROGS["A"] = _build_P1(mk())
        _PROGS["B"] = _build_L(mk())
        _PROGS["C"] = _build_L(mk())
        _PROGS["D"] = _build_D(mk())
    return _PROGS


def kernel(hidden_states, phi_w, psi_w, gat_lin_w, gat_att, wp, w1, b1, w2, b2,
           _profile=None):
    f32 = np.float32
    bf16 = ml_dtypes.bfloat16
    hidden_states = np.asarray(hidden_states, f32)
    progs = _get_progs()
    C = lambda a: np.ascontiguousarray(a)
    times = {}

    def run(tag, in_maps, core_ids):
        r = run_bass_kernel_spmd(progs[tag], in_maps, core_ids=core_ids)
        if _profile is not None:
            times[tag] = r.exec_time_ns
        return r.results

    glw = np.asarray(gat_lin_w, f32)
    ga = np.asarray(gat_att, f32)
    xTb = [C(hidden_states[b].T.astype(bf16)) for b in range(B)]

    # ---- launch P0: M = phi_w.T @ psi_w chunks, V = W^T [a_src|a_dst] ----
    # (reference einsum 'bsd,ed->bse' is x @ phi_w.T, so scores = x M x.T with
    # M = phi_w.T @ psi_w; the contraction runs over the e rows of both.)
    pT = np.asarray(phi_w, f32).astype(bf16)
    sT = C(np.asarray(psi_w, f32).astype(bf16))
    in_0 = []
    for c in range(8):
        l, hd = c // 4, c % 4
        in_0.append({
            "pTc": C(pT[:, c * 128:(c + 1) * 128]),
            "sT": sT,
            "Wn": C(glw[l, hd * H:(hd + 1) * H, :].astype(bf16)),
            "a2": C(np.stack([ga[l, hd, :H], ga[l, hd, H:]], axis=1).astype(bf16)),
        })
    r0 = run("A0", in_0, list(range(8)))
    Mfull = C(np.concatenate([r0[c]["Mc"] for c in range(8)], axis=0))
    V2 = [[C(r0[l * 4 + hd]["VT"].T.astype(bf16)) for hd in range(4)] for l in range(2)]

    # ---- launch P1: edge build ----
    in_a = []
    for c in range(8):
        b, rcn = c // 4, c % 4
        in_a.append({
            "xT": xTb[b], "xTc": C(xTb[b][:, rcn * CH:(rcn + 1) * CH]),
            "Mm": Mfull,
            "srcx": C(np.arange(rcn * CH, (rcn + 1) * CH, dtype=np.float32)[:, None]),
        })
    ra = run("A", in_a, list(range(8)))
    topi = np.stack([np.concatenate([ra[b * 4 + r]["topi"] for r in range(4)], 0) for b in range(B)])
    ew = np.stack([np.concatenate([ra[b * 4 + r]["ew"] for r in range(4)], 0) for b in range(B)])
    tpi16 = [C(topi[b].astype(np.int16)) for b in range(B)]
    ew16 = [C(ew[b].astype(bf16)) for b in range(B)]

    # ---- launches P2, P3: the two GAT layers (host pre-sums partials) ----
    xin = xTb
    for li, tag in enumerate(("B", "C")):
        in_l = []
        for c in range(8):
            b, hd = c // 4, c % 4
            in_l.append({
                "xT": xin[b],
                "WT": C(glw[li, hd * H:(hd + 1) * H, :].T.astype(bf16)),
                "V2": V2[li][hd],
                "tpi": tpi16[b], "ewd": ew16[b],
            })
        rl = run(tag, in_l, list(range(8)))
        xin = []
        for b in range(B):
            acc = sum(rl[b * 4 + i]["gT"].astype(f32) for i in range(4))
            xin.append(C(np.maximum(acc, 0.0).astype(bf16)))

    # ---- launch P4: pooling + projection head ----
    in_d = []
    for b in range(B):
        in_d.append({
            "x2T": xin[b],
            "wpc": C(np.asarray(wp, f32).reshape(H, 1).astype(bf16)),
            "w1T": C(np.asarray(w1, f32).T.astype(bf16)),
            "b1c": C(np.asarray(b1, f32)[:, None]),
            "w2T": C(np.asarray(w2, f32).T.astype(bf16)),
            "b2c": C(np.asarray(b2, f32)[:, None]),
        })
    rd = run("D", in_d, [0, 1])
    out = np.stack([rd[b]["res"][:, 0].astype(f32) for b in range(B)])
    if _profile is not None:
        _profile.update(times)
    return out


# revision 15
# speedup vs baseline: 1.4464x; 1.1073x over previous
"""Trainium2 Bass kernel for nn_GraphSemanticExtractor (GNN message passing).

Sharding (8 NeuronCores), 5 launches with host-side layout glue between them:
  P0: core c => 128-row chunk of M = phi_w @ psi_w.T, plus V = W^T [a_src|a_dst]
      for (layer l=c//4, head hd=c%4).
  P1 (edge build): core c => (batch b=c//4, 256-row chunk rc=c%4);
      scores = (x_c @ M) @ x.T, top-8, softmax over the 8, self-edge mask.
  P2/P3 (GAT layers 1/2): core c => (batch b=c//4, head hd=c%4); between the
      two, the host computes x1 = relu(sum of per-head partials).
  P4: pool + projection head, core b in {0,1}.

The sparse top-k aggregation out[dst] += wgt*h[src] is a dense matmul
out.T = h.T @ R with R[s,t] = ew_k(s)*exp(lrelu(e_src[s]+e_dst[t])) at
t=topi[s,k]; R is built by scattering ew into M0 (gpsimd local_scatter) and a
dense lrelu/exp of the rank-1 e-grid, all overlapped with the h matmul on PE.
"""

import sys

sys.path.insert(0, "/opt/trn_rl_repo")
sys.path.insert(0, "/opt/trn_rl_repo/concourse")

from contextlib import ExitStack

import ml_dtypes
import numpy as np

import concourse.bass as bass
import concourse.tile as tile
from concourse import bacc, mybir
from concourse.bass_utils import run_bass_kernel_spmd

F32 = mybir.dt.float32
BF16 = mybir.dt.bfloat16
U32 = mybir.dt.uint32
I16 = mybir.dt.int16
AF = mybir.ActivationFunctionType
OP = mybir.AluOpType
AX = mybir.AxisListType

B, S, H = 2, 1024, 1024
HEADS, K = 4, 8
SEM = 512
NB = H // 128  # 8 partition blocks
CH = S // 4    # 256 rows per edge-build core


def _build_P0(nc):
    """Per core: 128 rows of M = phi_w @ psi_w.T, and V = W^T [a_src|a_dst]
    for one (layer, head)."""
    pTc = nc.dram_tensor("pTc", [H, 128], BF16, kind="ExternalInput")
    sT = nc.dram_tensor("sT", [H, H], BF16, kind="ExternalInput")
    Wn = nc.dram_tensor("Wn", [H, H], BF16, kind="ExternalInput")
    a2 = nc.dram_tensor("a2", [H, 2], BF16, kind="ExternalInput")
    Mc = nc.dram_tensor("Mc", [128, H], BF16, kind="ExternalOutput")
    VT = nc.dram_tensor("VT", [2, H], F32, kind="ExternalOutput")

    with tile.TileContext(nc) as tc, ExitStack() as ctx:
        pers = ctx.enter_context(tc.tile_pool(name="pers", bufs=1))
        psum = ctx.enter_context(tc.tile_pool(name="psum", bufs=4, space="PSUM"))

        # all input DMAs on the sync queue, smallest first (single DMA engine
        # processes FIFO; a big transfer queued first would stall the rest)
        a2t = pers.tile([128, NB, 2], BF16, tag="a2t")
        nc.sync.dma_start(out=a2t[:], in_=a2[:].rearrange("(kb p) c -> p kb c", p=128))
        pT16 = pers.tile([128, NB, 128], BF16, tag="pT16")
        nc.sync.dma_start(out=pT16[:], in_=pTc[:].rearrange("(kb p) c -> p kb c", p=128))
        sT16 = pers.tile([128, NB, H], BF16, tag="sT16")
        nc.sync.dma_start(out=sT16[:], in_=sT[:].rearrange("(kb p) s -> p kb s", p=128))
        Wn16 = pers.tile([128, NB, H], BF16, tag="Wn16")
        nc.sync.dma_start(out=Wn16[:], in_=Wn[:].rearrange("(kb p) s -> p kb s", p=128))

        Mc16 = pers.tile([128, H], BF16, tag="Mc16")
        for n0 in range(0, H, 512):
            pt = psum.tile([128, 512], F32, tag="mm")
            for k in range(NB):
                nc.tensor.matmul(pt[:], pT16[:, k, :], sT16[:, k, n0:n0 + 512],
                                 start=(k == 0), stop=(k == NB - 1))
            nc.vector.tensor_copy(out=Mc16[:, n0:n0 + 512], in_=pt[:])
        nc.sync.dma_start(out=Mc[:], in_=Mc16[:])

        Vt = pers.tile([2, H], F32, tag="Vt")
        for n0 in range(0, H, 512):
            pt = psum.tile([2, 512], F32, tag="vm")
            for k in range(NB):
                nc.tensor.matmul(pt[:], a2t[:, k, :], Wn16[:, k, n0:n0 + 512],
                                 start=(k == 0), stop=(k == NB - 1))
            nc.vector.tensor_copy(out=Vt[:, n0:n0 + 512], in_=pt[:])
        nc.sync.dma_start(out=VT[:], in_=Vt[:])
    nc.compile()
    return nc


def _build_P1(nc):
    """Edge build: scores = (x_c @ M) @ x.T, top-8 + softmax + self-mask."""
    xT = nc.dram_tensor("xT", [H, S], BF16, kind="ExternalInput")
    xTc = nc.dram_tensor("xTc", [H, CH], BF16, kind="ExternalInput")
    Mm = nc.dram_tensor("Mm", [H, H], BF16, kind="ExternalInput")
    srcx = nc.dram_tensor("srcx", [CH, 1], F32, kind="ExternalInput")
    topi = nc.dram_tensor("topi", [CH, K], U32, kind="ExternalOutput")
    ew = nc.dram_tensor("ew", [CH, K], F32, kind="ExternalOutput")

    with tile.TileContext(nc) as tc, ExitStack() as ctx:
        pers = ctx.enter_context(tc.tile_pool(name="pers", bufs=1))
        psum = ctx.enter_context(tc.tile_pool(name="psum", bufs=4, space="PSUM"))
        psumb = ctx.enter_context(tc.tile_pool(name="psumb", bufs=4, space="PSUM"))

        sx = pers.tile([128, 2, 1], F32, tag="sx")
        nc.sync.dma_start(out=sx[:], in_=srcx[:].rearrange("(m p) c -> p m c", p=128))
        xTc16 = pers.tile([128, NB, CH], BF16, tag="xTc16")
        nc.sync.dma_start(out=xTc16[:], in_=xTc[:].rearrange("(kb p) s -> p kb s", p=128))
        M16 = pers.tile([128, NB, H], BF16, tag="M16")
        nc.sync.dma_start(out=M16[:], in_=Mm[:].rearrange("(kb p) s -> p kb s", p=128))
        xT16 = pers.tile([128, NB, S], BF16, tag="xT16")
        nc.sync.dma_start(out=xT16[:], in_=xT[:].rearrange("(kb p) s -> p kb s", p=128))

        # PT[j, s-chunk] = (x_c @ M).T  (j = feature of M's column space)
        PT16 = pers.tile([128, NB, CH], BF16, tag="PT16")
        for m in range(NB):
            pt = psumb.tile([128, CH], F32, tag="ptm")
            for k in range(NB):
                nc.tensor.matmul(pt[:], M16[:, k, m * 128:(m + 1) * 128], xTc16[:, k, :],
                                 start=(k == 0), stop=(k == NB - 1))
            nc.vector.tensor_copy(out=PT16[:, m, :], in_=pt[:])

        # scores [s-chunk, t] f32; top-8 of sb-block 0 overlaps sb-block 1's matmuls
        sc = pers.tile([128, 2, S], F32, tag="scores")
        mv = pers.tile([128, 2, K], F32, tag="mv")
        ti = pers.tile([128, 2, K], U32, tag="ti")
        for sb in range(2):
            for n0 in range(0, S, 512):
                pt = psum.tile([128, 512], F32, tag="scm")
                for k in range(NB):
                    nc.tensor.matmul(pt[:], PT16[:, k, sb * 128:(sb + 1) * 128],
                                     xT16[:, k, n0:n0 + 512],
                                     start=(k == 0), stop=(k == NB - 1))
                nc.vector.tensor_copy(out=sc[:, sb, n0:n0 + 512], in_=pt[:])
            nc.vector.max(mv[:, sb, :], sc[:, sb, :])
            nc.vector.max_index(ti[:, sb, :], mv[:, sb, :], sc[:, sb, :])
        ex = pers.tile([128, 2, K], F32, tag="ex")
        nc.scalar.activation(ex[:], mv[:], AF.Exp)
        sm = pers.tile([128, 2, 1], F32, tag="sm")
        nc.vector.tensor_reduce(sm[:], ex[:], axis=AX.X, op=OP.add)
        nc.vector.tensor_scalar(sm[:], sm[:], 1e-8, None, op0=OP.add)
        rc = pers.tile([128, 2, 1], F32, tag="rc")
        nc.vector.reciprocal(rc[:], sm[:])
        tif = pers.tile([128, 2, K], F32, tag="tif")
        nc.vector.tensor_copy(out=tif[:], in_=ti[:])
        w8 = pers.tile([128, 2, K], F32, tag="w8")
        msk = pers.tile([128, 2, K], F32, tag="msk")
        for m in range(2):
            nc.vector.tensor_scalar(w8[:, m, :], ex[:, m, :], rc[:, m, :], 1e-8, op0=OP.mult, op1=OP.max)
            nc.vector.tensor_scalar(msk[:, m, :], tif[:, m, :], sx[:, m, :], None, op0=OP.is_equal)
            nc.vector.tensor_scalar(msk[:, m, :], msk[:, m, :], -1.0, 1.0, op0=OP.mult, op1=OP.add)
        ewt = pers.tile([128, 2, K], F32, tag="ewt")
        nc.vector.tensor_tensor(ewt[:], w8[:], msk[:], op=OP.mult)
        nc.sync.dma_start(out=topi[:].rearrange("(m p) k -> p m k", p=128), in_=ti[:])
        nc.sync.dma_start(out=ew[:].rearrange("(m p) k -> p m k", p=128), in_=ewt[:])
    nc.compile()
    return nc


def _build_L(nc):
    """One GAT layer for one (batch, head).  gT[feat, node] = (agg/attn)/HEADS."""
    xT = nc.dram_tensor("xT", [H, S], BF16, kind="ExternalInput")
    WT = nc.dram_tensor("WT", [H, H], BF16, kind="ExternalInput")
    V2 = nc.dram_tensor("V2", [H, 2], BF16, kind="ExternalInput")
    tpi = nc.dram_tensor("tpi", [S, K], I16, kind="ExternalInput")
    ewd = nc.dram_tensor("ewd", [S, K], BF16, kind="ExternalInput")
    gT = nc.dram_tensor("gT", [H, S], BF16, kind="ExternalOutput")

    with tile.TileContext(nc) as tc, ExitStack() as ctx:
        pers = ctx.enter_context(tc.tile_pool(name="pers", bufs=1))
        psum = ctx.enter_context(tc.tile_pool(name="psum", bufs=4, space="PSUM"))
        psmall = ctx.enter_context(tc.tile_pool(name="psmall", bufs=1, space="PSUM"))

        # all inputs on the sync queue, smallest first (FIFO DMA engine)
        V16 = pers.tile([128, NB, 2], BF16, tag="V16")
        nc.sync.dma_start(out=V16[:], in_=V2[:].rearrange("(kb p) c -> p kb c", p=128))
        tpw = pers.tile([128, NB, K], I16, tag="tpw")
        nc.sync.dma_start(out=tpw[:], in_=tpi[:].rearrange("(m p) k -> p m k", p=128))
        ews16 = pers.tile([128, NB, K], BF16, tag="ews16")
        nc.sync.dma_start(out=ews16[:], in_=ewd[:].rearrange("(m p) k -> p m k", p=128))
        xT16 = pers.tile([128, NB, S], BF16, tag="xT16")
        nc.sync.dma_start(out=xT16[:], in_=xT[:].rearrange("(kb p) s -> p kb s", p=128))
        WT16 = pers.tile([128, NB, H], BF16, tag="WT16")
        nc.sync.dma_start(out=WT16[:], in_=WT[:].rearrange("(kb p) s -> p kb s", p=128))

        # gpsimd: M0 blocks (scatter of ew into dense [s, t])
        M0 = pers.tile([128, NB, S], BF16, tag="M0")
        for m in range(NB):
            nc.gpsimd.local_scatter(M0[:, m, :], ews16[:, m, :], tpw[:, m, :],
                                    channels=128, num_elems=S, num_idxs=K)

        # PE: e_srcT / e_dstT [1, node] = (V col)^T x  (separate 1-row outputs so
        # each lands at base partition 0)
        esT = pers.tile([1, S], F32, tag="esT")
        edT = pers.tile([1, S], F32, tag="edT")
        for col, dst in ((1, edT), (0, esT)):
            for n0 in range(0, S, 512):
                pt = psmall.tile([1, 512], F32, tag="ebp")
                for k in range(NB):
                    nc.tensor.matmul(pt[:], V16[:, k, col:col + 1], xT16[:, k, n0:n0 + 512],
                                     start=(k == 0), stop=(k == NB - 1))
                nc.vector.tensor_copy(out=dst[:, n0:n0 + 512], in_=pt[:])

        # PE: broadcast e_dst across partitions (rank-1 matmul with ones)
        ones1r = pers.tile([1, 128], F32, tag="ones1r")
        nc.vector.memset(ones1r[:], 1.0)
        edb = pers.tile([128, S], F32, tag="edb")
        for n0 in range(0, S, 512):
            pt = psmall.tile([128, 512], F32, tag="edbp")
            nc.tensor.matmul(pt[:], ones1r[:], edT[0:1, n0:n0 + 512], start=True, stop=True)
            nc.scalar.copy(out=edb[:, n0:n0 + 512], in_=pt[:])

        # PE: e_src into partitions (transpose via 1-col matmul)
        ones11 = pers.tile([1, 1], F32, tag="ones11")
        nc.vector.memset(ones11[:], 1.0)
        esc = pers.tile([128, NB, 1], F32, tag="esc")
        for m in range(NB):
            pt = psmall.tile([128, 1], F32, tag="escp")
            nc.tensor.matmul(pt[:], esT[0:1, m * 128:(m + 1) * 128], ones11[:],
                             start=True, stop=True)
            nc.vector.tensor_copy(out=esc[:, m, :], in_=pt[:])

        # PE: h [node, feat] bf16 (evictions on DVE only, Act stays free for R)
        h16 = pers.tile([128, NB, H], BF16, tag="h16")
        for m in range(NB):
            for n0 in range(0, H, 512):
                pt = psum.tile([128, 512], F32, tag="mmp")
                for k in range(NB):
                    nc.tensor.matmul(pt[:], xT16[:, k, m * 128:(m + 1) * 128],
                                     WT16[:, k, n0:n0 + 512],
                                     start=(k == 0), stop=(k == NB - 1))
                nc.vector.tensor_copy(out=h16[:, m, n0:n0 + 512], in_=pt[:])

        # Act: all lrelu then all exp (2 act-table loads total); DVE: R = M0 * ez
        zl8 = pers.tile([128, NB, S], BF16, tag="zl8")
        for m in range(NB):
            nc.scalar.activation(zl8[:, m, :], edb[:], AF.Lrelu, bias=esc[:, m, :], alpha=0.2)
        ez8 = pers.tile([128, NB, S], BF16, tag="ez8")
        for m in range(NB):
            nc.scalar.activation(ez8[:, m, :], zl8[:, m, :], AF.Exp)
        R = pers.tile([128, NB, S], BF16, tag="R")
        for m in range(NB):
            nc.vector.tensor_tensor(R[:, m, :], M0[:, m, :], ez8[:, m, :], op=OP.mult)

        # PE: attn^T [1, t] = 1^T R ; arc = 0.25 / (attn + 1e-8)
        onesc = pers.tile([128, 1], BF16, tag="onesc")
        nc.vector.memset(onesc[:], 1.0)
        atT = pers.tile([1, S], F32, tag="atT")
        for n0 in range(0, S, 512):
            pt = psmall.tile([1, 512], F32, tag="atp")
            for k in range(NB):
                nc.tensor.matmul(pt[:], onesc[:], R[:, k, n0:n0 + 512],
                                 start=(k == 0), stop=(k == NB - 1))
            nc.vector.tensor_copy(out=atT[:, n0:n0 + 512], in_=pt[:])
        nc.vector.tensor_scalar(atT[:], atT[:], 1e-8, None, op0=OP.add)
        arc = pers.tile([1, S], F32, tag="arc")
        nc.vector.reciprocal(arc[:], atT[:])
        nc.vector.tensor_scalar(arc[:], arc[:], 1.0 / HEADS, None, op0=OP.mult)
        rcb = pers.tile([128, S], F32, tag="rcb")
        nc.gpsimd.partition_broadcast(rcb[:], arc[:])

        # PE: out^T [feat, t] = h^T R, scaled by rcb; chunked DMA out
        gsb = pers.tile([128, NB, S], BF16, tag="gsb")
        gTr = gT[:].rearrange("(m p) t -> p m t", p=128)
        for m in range(NB):
            for n0 in range(0, S, 512):
                pt = psum.tile([128, 512], F32, tag="mmp")
                for k in range(NB):
                    nc.tensor.matmul(pt[:], h16[:, k, m * 128:(m + 1) * 128],
                                     R[:, k, n0:n0 + 512],
                                     start=(k == 0), stop=(k == NB - 1))
                nc.vector.tensor_tensor(gsb[:, m, n0:n0 + 512], pt[:], rcb[:, n0:n0 + 512], op=OP.mult)
            nc.sync.dma_start(out=gTr[:, m, :], in_=gsb[:, m, :])
    nc.compile()
    return nc


def _build_D(nc):
    """Attention pool over nodes + 2-layer projection head, one batch per core."""
    x2T = nc.dram_tensor("x2T", [H, S], BF16, kind="ExternalInput")
    wpc = nc.dram_tensor("wpc", [H, 1], BF16, kind="ExternalInput")
    w1T = nc.dram_tensor("w1T", [H, SEM], BF16, kind="ExternalInput")
    b1c = nc.dram_tensor("b1c", [SEM, 1], F32, kind="ExternalInput")
    w2T = nc.dram_tensor("w2T", [SEM, SEM], BF16, kind="ExternalInput")
    b2c = nc.dram_tensor("b2c", [SEM, 1], F32, kind="ExternalInput")
    res = nc.dram_tensor("res", [SEM, 1], F32, kind="ExternalOutput")

    with tile.TileContext(nc) as tc, ExitStack() as ctx:
        pers = ctx.enter_context(tc.tile_pool(name="pers", bufs=1))
        tmp = ctx.enter_context(tc.tile_pool(name="tmp", bufs=3))
        psum = ctx.enter_context(tc.tile_pool(name="psum", bufs=3, space="PSUM"))

        wp16 = pers.tile([128, NB, 1], BF16, tag="wp16")
        nc.sync.dma_start(out=wp16[:], in_=wpc[:].rearrange("(kb p) c -> p kb c", p=128))
        b1f = pers.tile([128, 4, 1], F32, tag="b1f")
        nc.sync.dma_start(out=b1f[:], in_=b1c[:].rearrange("(m p) c -> p m c", p=128))
        b2f = pers.tile([128, 4, 1], F32, tag="b2f")
        nc.sync.dma_start(out=b2f[:], in_=b2c[:].rearrange("(m p) c -> p m c", p=128))
        # x2T column-chunked so psc starts after the first half arrives
        x3T = pers.tile([128, NB, S], BF16, tag="x3T")
        x2Tr = x2T[:].rearrange("(kb p) s -> p kb s", p=128)
        for n0 in range(0, S, 512):
            nc.sync.dma_start(out=x3T[:, :, n0:n0 + 512], in_=x2Tr[:, :, n0:n0 + 512])
        w116 = pers.tile([128, NB, SEM], BF16, tag="w116")
        nc.sync.dma_start(out=w116[:], in_=w1T[:].rearrange("(kb p) c -> p kb c", p=128))
        w216 = pers.tile([128, 4, SEM], BF16, tag="w216")
        nc.sync.dma_start(out=w216[:], in_=w2T[:].rearrange("(kb p) c -> p kb c", p=128))

        psc = pers.tile([1, S], F32, tag="psc")
        for n0 in range(0, S, 512):
            pt = psum.tile([1, 512], F32, tag="sp")
            for k in range(NB):
                nc.tensor.matmul(pt[:], wp16[:, k, :], x3T[:, k, n0:n0 + 512],
                                 start=(k == 0), stop=(k == NB - 1))
            nc.vector.tensor_copy(out=psc[:, n0:n0 + 512], in_=pt[:])

        mx = pers.tile([1, 1], F32, tag="mx")
        nc.vector.tensor_reduce(mx[:], psc[:], axis=AX.X, op=OP.max)
        nmx = pers.tile([1, 1], F32, tag="nmx")
        nc.vector.tensor_scalar(nmx[:], mx[:], -1.0, None, op0=OP.mult)
        ev = pers.tile([1, S], F32, tag="ev")
        nc.scalar.activation(ev[:], psc[:], AF.Exp, bias=nmx[:])
        sm = pers.tile([1, 1], F32, tag="sm")
        nc.vector.tensor_reduce(sm[:], ev[:], axis=AX.X, op=OP.add)
        rc = pers.tile([1, 1], F32, tag="rc")
        nc.vector.reciprocal(rc[:], sm[:])
        alT = pers.tile([1, S], BF16, tag="alT")
        nc.vector.tensor_scalar(alT[:], ev[:], rc[:], None, op0=OP.mult)

        alb = pers.tile([128, S], BF16, tag="alb")
        nc.gpsimd.partition_broadcast(alb[:], alT[:])
        pldf = pers.tile([128, NB, 1], F32, tag="pldf")
        pld = pers.tile([128, NB, 1], BF16, tag="pld")
        for m in range(NB):
            junk = tmp.tile([128, S], BF16, tag="junk")
            nc.vector.scalar_tensor_tensor(junk[:], x3T[:, m, :], 1.0, alb[:],
                                           op0=OP.mult, op1=OP.mult,
                                           accum_out=pldf[:, m, :])
        nc.vector.tensor_copy(out=pld[:], in_=pldf[:])

        hid = pers.tile([128, 4, 1], BF16, tag="hid")
        for m in range(4):
            pt = psum.tile([128, 1], F32, tag="sp1")
            for k in range(NB):
                nc.tensor.matmul(pt[:], w116[:, k, m * 128:(m + 1) * 128], pld[:, k, :],
                                 start=(k == 0), stop=(k == NB - 1))
            nc.scalar.activation(hid[:, m, :], pt[:], AF.Relu, bias=b1f[:, m, :])

        rsb = pers.tile([128, 4, 1], F32, tag="rsb")
        for m in range(4):
            pt = psum.tile([128, 1], F32, tag="sp1")
            for k in range(4):
                nc.tensor.matmul(pt[:], w216[:, k, m * 128:(m + 1) * 128], hid[:, k, :],
                                 start=(k == 0), stop=(k == 3))
            nc.vector.tensor_tensor(rsb[:, m, :], pt[:], b2f[:, m, :], op=OP.add)
        nc.sync.dma_start(out=res[:].rearrange("(m p) c -> p m c", p=128), in_=rsb[:])
    nc.compile()
    return nc


_PROGS = {}


def _get_progs():
    if not _PROGS:
        def mk():
            return bacc.Bacc("TRN2", target_bir_lowering=False, debug=False,
                             enable_asserts=True, num_devices=8)
        _PROGS["A0"] = _build_P0(mk())
        _PROGS["A"] = _build_P1(mk())
        _PROGS["B"] = _build_L(mk())
        _PROGS["C"] = _build_L(mk())
        _PROGS["D"] = _build_D(mk())
    return _PROGS


def kernel(hidden_states, phi_w, psi_w, gat_lin_w, gat_att, wp, w1, b1, w2, b2,
           _profile=None):
    f32 = np.float32
    bf16 = ml_dtypes.bfloat16
    hidden_states = np.asarray(hidden_states, f32)
    progs = _get_progs()
    C = lambda a: np.ascontiguousarray(a)
    times = {}

    def run(tag, in_maps, core_ids):
        r = run_bass_kernel_spmd(progs[tag], in_maps, core_ids=core_ids)
        if _profile is not None:
            times[tag] = r.exec_time_ns
        return r.results

    glw = np.asarray(gat_lin_w, f32)
    ga = np.asarray(gat_att, f32)
    xTb = [C(hidden_states[b].T.astype(bf16)) for b in range(B)]

    # ---- launch P0: M = phi_w.T @ psi_w chunks, V = W^T [a_src|a_dst] ----
    # (reference einsum 'bsd,ed->bse' is x @ phi_w.T, so scores = x M x.T with
    # M = phi_w.T @ psi_w; the contraction runs over the e rows of both.)
    pT = np.asarray(phi_w, f32).astype(bf16)
    sT = C(np.asarray(psi_w, f32).astype(bf16))
    in_0 = []
    for c in range(8):
        l, hd = c // 4, c % 4
        in_0.append({
            "pTc": C(pT[:, c * 128:(c + 1) * 128]),
            "sT": sT,
            "Wn": C(glw[l, hd * H:(hd + 1) * H, :].astype(bf16)),
            "a2": C(np.stack([ga[l, hd, :H], ga[l, hd, H:]], axis=1).astype(bf16)),
        })
    r0 = run("A0", in_0, list(range(8)))
    Mfull = C(np.concatenate([r0[c]["Mc"] for c in range(8)], axis=0))
    V2 = [[C(r0[l * 4 + hd]["VT"].T.astype(bf16)) for hd in range(4)] for l in range(2)]

    # ---- launch P1: edge build ----
    in_a = []
    for c in range(8):
        b, rcn = c // 4, c % 4
        in_a.append({
            "xT": xTb[b], "xTc": C(xTb[b][:, rcn * CH:(rcn + 1) * CH]),
            "Mm": Mfull,
            "srcx": C(np.arange(rcn * CH, (rcn + 1) * CH, dtype=np.float32)[:, None]),
        })
    ra = run("A", in_a, list(range(8)))
    topi = np.stack([np.concatenate([ra[b * 4 + r]["topi"] for r in range(4)], 0) for b in range(B)])
    ew = np.stack([np.concatenate([ra[b * 4 + r]["ew"] for r in range(4)], 0) for b in range(B)])
    tpi16 = [C(topi[b].astype(np.int16)) for b in range(B)]
    ew16 = [C(ew[b].astype(bf16)) for b in range(B)]

    # ---- launches P2, P3: the two GAT layers (host pre-sums partials) ----
    xin = xTb
    for li, tag in enumerate(("B", "C")):
        in_l = []
        for c in range(8):
            b, hd = c // 4, c % 4
            in_l.append({
                "xT": xin[b],
                "WT": C(glw[li, hd * H:(hd + 1) * H, :].T.astype(bf16)),
                "V2": V2[li][hd],
                "tpi": tpi16[b], "ewd": ew16[b],
            })
        rl = run(tag, in_l, list(range(8)))
        xin = []
        for b in range(B):
            acc = sum(rl[b * 4 + i]["gT"].astype(f32) for i in range(4))
            xin.append(C(np.maximum(acc, 0.0).astype(bf16)))

    # ---- launch P4: pooling + projection head ----
    in_d = []
    for b in range(B):
        in_d.append({
            "x2T": xin[b],
            "wpc": C(np.asarray(wp, f32).reshape(H, 1).astype(bf16)),
            "w1T": C(np.asarray(w1, f32).T.astype(bf16)),
            "b1c": C(np.asarray(b1, f32)[:, None]),
            "w2T": C(np.asarray(w2, f32).T.astype(bf16)),
            "b2c": C(np.asarray(b2, f32)[:, None]),
        })
    rd = run("D", in_d, [0, 1])
    out = np.stack([rd[b]["res"][:, 0].astype(f32) for b in range(B)])
    if _profile is not None:
        _profile.update(times)
    return out


# revision 36
# speedup vs baseline: 1.5355x; 1.0617x over previous
"""Trainium2 Bass kernel for nn_GraphSemanticExtractor (GNN message passing).

Sharding (8 NeuronCores), 5 launches with host-side layout glue between them:
  P0: core c => 128-row chunk of M = phi_w @ psi_w.T, plus V = W^T [a_src|a_dst]
      for (layer l=c//4, head hd=c%4).
  P1 (edge build): core c => (batch b=c//4, 256-row chunk rc=c%4);
      scores = (x_c @ M) @ x.T, top-8, softmax over the 8, self-edge mask.
  P2/P3 (GAT layers 1/2): core c => (batch b=c//4, head hd=c%4); between the
      two, the host computes x1 = relu(sum of per-head partials).
  P4: pool + projection head, core b in {0,1}.

The sparse top-k aggregation out[dst] += wgt*h[src] is a dense matmul
out.T = h.T @ R with R[s,t] = ew_k(s)*exp(lrelu(e_src[s]+e_dst[t])) at
t=topi[s,k]; R is built by scattering ew into M0 (gpsimd local_scatter) and a
dense lrelu/exp of the rank-1 e-grid, all overlapped with the h matmul on PE.
"""

import sys

sys.path.insert(0, "/opt/trn_rl_repo")
sys.path.insert(0, "/opt/trn_rl_repo/concourse")

from contextlib import ExitStack

import ml_dtypes
import numpy as np

import concourse.bass as bass
import concourse.tile as tile
from concourse import bacc, mybir
from concourse.bass_utils import run_bass_kernel_spmd

F32 = mybir.dt.float32
BF16 = mybir.dt.bfloat16
F8 = mybir.dt.float8e4
U32 = mybir.dt.uint32
I16 = mybir.dt.int16
AF = mybir.ActivationFunctionType
OP = mybir.AluOpType
AX = mybir.AxisListType
DR = mybir.MatmulPerfMode.DoubleRow

B, S, H = 2, 1024, 1024
HEADS, K = 4, 8
SEM = 512
NB = H // 128  # 8 partition blocks
ND = NB // 2   # 4 double-row blocks for fp8 DoubleRow matmuls
CH = S // 4    # 256 rows per edge-build core

# fp8 e4m3 has min-normal 2^-6; the tiny GAT weights (~0.02 scale) are scaled
# up on the host and the factors folded back into on-device scalars.
W_SCALE = 32.0   # W.T fed to the h matmul
V_SCALE = 64.0   # V = W^T [a_src|a_dst] fed to the e matmuls
A_SCALE = 64.0   # gat_att halves fed to P0's V matmul


def _build_P0(nc):
    """Per core: 128 rows of M = phi_w.T @ psi_w, and V = W^T [a_src|a_dst]
    for one (layer, head).  The V matmul runs in fp8 DoubleRow (host scales
    its inputs by A_SCALE*W_SCALE; the eviction scales back)."""
    pTc = nc.dram_tensor("pTc", [H, 128], BF16, kind="ExternalInput")
    sT = nc.dram_tensor("sT", [H, H], BF16, kind="ExternalInput")
    Wn = nc.dram_tensor("Wn", [H, H], F8, kind="ExternalInput")
    a2 = nc.dram_tensor("a2", [H, 2], F8, kind="ExternalInput")
    Mc = nc.dram_tensor("Mc", [128, H], BF16, kind="ExternalOutput")
    VT = nc.dram_tensor("VT", [2, H], F32, kind="ExternalOutput")

    with tile.TileContext(nc) as tc, ExitStack() as ctx:
        pers = ctx.enter_context(tc.tile_pool(name="pers", bufs=1))
        psum = ctx.enter_context(tc.tile_pool(name="psum", bufs=4, space="PSUM"))

        # all input DMAs on the sync queue, smallest first (single DMA engine
        # processes FIFO; a big transfer queued first would stall the rest)
        a2t = pers.tile([128, NB, 2], F8, tag="a2t")
        nc.sync.dma_start(out=a2t[:], in_=a2[:].rearrange("(kb p) c -> p kb c", p=128))
        pT16 = pers.tile([128, NB, 128], BF16, tag="pT16")
        nc.sync.dma_start(out=pT16[:], in_=pTc[:].rearrange("(kb p) c -> p kb c", p=128))
        Wn16 = pers.tile([128, NB, H], F8, tag="Wn16")
        nc.sync.dma_start(out=Wn16[:], in_=Wn[:].rearrange("(kb p) s -> p kb s", p=128))
        sT16 = pers.tile([128, NB, H], BF16, tag="sT16")
        nc.sync.dma_start(out=sT16[:], in_=sT[:].rearrange("(kb p) s -> p kb s", p=128))

        Vt = pers.tile([2, H], F32, tag="Vt")
        for n0 in range(0, H, 512):
            pt = psum.tile([2, 512], F32, tag="vm")
            for k in range(NB):
                nc.tensor.matmul(pt[:], a2t[:, k, :], Wn16[:, k, n0:n0 + 512],
                                 start=(k == 0), stop=(k == NB - 1))
            nc.vector.tensor_scalar(Vt[:, n0:n0 + 512], pt[:],
                                    1.0 / (A_SCALE * W_SCALE), None, op0=OP.mult)
        nc.sync.dma_start(out=VT[:], in_=Vt[:])

        Mc16 = pers.tile([128, H], BF16, tag="Mc16")
        for n0 in range(0, H, 512):
            pt = psum.tile([128, 512], F32, tag="mm")
            for k in range(NB):
                nc.tensor.matmul(pt[:], pT16[:, k, :], sT16[:, k, n0:n0 + 512],
                                 start=(k == 0), stop=(k == NB - 1))
            nc.vector.tensor_copy(out=Mc16[:, n0:n0 + 512], in_=pt[:])
        nc.sync.dma_start(out=Mc[:], in_=Mc16[:])
    nc.compile()
    return nc


def _build_P1(nc):
    """Edge build: scores = (x_c @ M) @ x.T, top-8 + softmax + self-mask."""
    xT = nc.dram_tensor("xT", [H, S], BF16, kind="ExternalInput")
    xTc = nc.dram_tensor("xTc", [H, CH], BF16, kind="ExternalInput")
    Mm = nc.dram_tensor("Mm", [H, H], BF16, kind="ExternalInput")
    srcx = nc.dram_tensor("srcx", [CH, 1], F32, kind="ExternalInput")
    topi = nc.dram_tensor("topi", [CH, K], U32, kind="ExternalOutput")
    ew = nc.dram_tensor("ew", [CH, K], F32, kind="ExternalOutput")

    with tile.TileContext(nc) as tc, ExitStack() as ctx:
        pers = ctx.enter_context(tc.tile_pool(name="pers", bufs=1))
        psum = ctx.enter_context(tc.tile_pool(name="psum", bufs=4, space="PSUM"))
        psumb = ctx.enter_context(tc.tile_pool(name="psumb", bufs=4, space="PSUM"))

        sx = pers.tile([128, 2, 1], F32, tag="sx")
        nc.sync.dma_start(out=sx[:], in_=srcx[:].rearrange("(m p) c -> p m c", p=128))
        xTc16 = pers.tile([128, NB, CH], BF16, tag="xTc16")
        nc.sync.dma_start(out=xTc16[:], in_=xTc[:].rearrange("(kb p) s -> p kb s", p=128))
        M16 = pers.tile([128, NB, H], BF16, tag="M16")
        nc.sync.dma_start(out=M16[:], in_=Mm[:].rearrange("(kb p) s -> p kb s", p=128))
        xT16 = pers.tile([128, NB, S], BF16, tag="xT16")
        nc.sync.dma_start(out=xT16[:], in_=xT[:].rearrange("(kb p) s -> p kb s", p=128))

        # preload the Exp act table while DMAs run so the top-k chain's exp
        # doesn't pay the 1.3us table load
        warm = pers.tile([1, 1], F32, tag="warm")
        nc.vector.memset(warm[:], 0.0)
        nc.scalar.activation(warm[:], warm[:], AF.Exp)

        # PT[j, s-chunk] = (x_c @ M).T  (j = feature of M's column space)
        PT16 = pers.tile([128, NB, CH], BF16, tag="PT16")
        for m in range(NB):
            pt = psumb.tile([128, CH], F32, tag="ptm")
            for k in range(NB):
                nc.tensor.matmul(pt[:], M16[:, k, m * 128:(m + 1) * 128], xTc16[:, k, :],
                                 start=(k == 0), stop=(k == NB - 1))
            nc.vector.tensor_copy(out=PT16[:, m, :], in_=pt[:])

        # scores [s-chunk, t] f32; the whole per-sb top-8/softmax/mask chain runs
        # while the other sb-block's matmuls occupy PE
        sc = pers.tile([128, 2, S], F32, tag="scores")
        mv = pers.tile([128, 2, K], F32, tag="mv")
        ti = pers.tile([128, 2, K], U32, tag="ti")
        ex = pers.tile([128, 2, K], F32, tag="ex")
        sm = pers.tile([128, 2, 1], F32, tag="sm")
        rc = pers.tile([128, 2, 1], F32, tag="rc")
        tif = pers.tile([128, 2, K], F32, tag="tif")
        w8 = pers.tile([128, 2, K], F32, tag="w8")
        msk = pers.tile([128, 2, K], F32, tag="msk")
        ewt = pers.tile([128, 2, K], F32, tag="ewt")
        topir = topi[:].rearrange("(m p) k -> p m k", p=128)
        ewr = ew[:].rearrange("(m p) k -> p m k", p=128)
        for sb in range(2):
            for n0 in range(0, S, 512):
                pt = psum.tile([128, 512], F32, tag="scm")
                for k in range(NB):
                    nc.tensor.matmul(pt[:], PT16[:, k, sb * 128:(sb + 1) * 128],
                                     xT16[:, k, n0:n0 + 512],
                                     start=(k == 0), stop=(k == NB - 1))
                nc.vector.tensor_copy(out=sc[:, sb, n0:n0 + 512], in_=pt[:])
            nc.vector.max(mv[:, sb, :], sc[:, sb, :])
            nc.vector.max_index(ti[:, sb, :], mv[:, sb, :], sc[:, sb, :])
            nc.sync.dma_start(out=topir[:, sb, :], in_=ti[:, sb, :])
            nc.scalar.activation(ex[:, sb, :], mv[:, sb, :], AF.Exp)
            nc.vector.tensor_reduce(sm[:, sb, :], ex[:, sb, :], axis=AX.X, op=OP.add)
            nc.vector.tensor_scalar(sm[:, sb, :], sm[:, sb, :], 1e-8, None, op0=OP.add)
            nc.vector.reciprocal(rc[:, sb, :], sm[:, sb, :])
            nc.vector.tensor_copy(out=tif[:, sb, :], in_=ti[:, sb, :])
            nc.vector.tensor_scalar(w8[:, sb, :], ex[:, sb, :], rc[:, sb, :], 1e-8, op0=OP.mult, op1=OP.max)
            nc.vector.tensor_scalar(msk[:, sb, :], tif[:, sb, :], sx[:, sb, :], None, op0=OP.is_equal)
            nc.vector.tensor_scalar(msk[:, sb, :], msk[:, sb, :], -1.0, 1.0, op0=OP.mult, op1=OP.add)
            nc.vector.tensor_tensor(ewt[:, sb, :], w8[:, sb, :], msk[:, sb, :], op=OP.mult)
            nc.sync.dma_start(out=ewr[:, sb, :], in_=ewt[:, sb, :])
    nc.compile()
    return nc


def _build_L(nc):
    """One GAT layer for one (batch, head).  gT[feat, node] = (agg/attn)/HEADS.
    The h matmul runs fp8 DoubleRow with W split into an fp8 hi/lo residual
    pair (hi + lo/16 restores ~bf16 weight accuracy; fp8 W alone costs 2e-2
    output error).  x and V are plain fp8 (~2e-3 each).  R and the
    aggregation stay bf16: fp8 R alone costs 4e-2."""
    xT = nc.dram_tensor("xT", [H, S], F8, kind="ExternalInput")
    WTh = nc.dram_tensor("WTh", [H, H], F8, kind="ExternalInput")
    WTl = nc.dram_tensor("WTl", [H, H], F8, kind="ExternalInput")
    V2 = nc.dram_tensor("V2", [H, 2], F8, kind="ExternalInput")
    tpi = nc.dram_tensor("tpi", [S, K], I16, kind="ExternalInput")
    ewd = nc.dram_tensor("ewd", [S, K], BF16, kind="ExternalInput")
    gT = nc.dram_tensor("gT", [H, S], BF16, kind="ExternalOutput")

    with tile.TileContext(nc) as tc, ExitStack() as ctx:
        pers = ctx.enter_context(tc.tile_pool(name="pers", bufs=1))
        psum = ctx.enter_context(tc.tile_pool(name="psum", bufs=4, space="PSUM"))
        psmall = ctx.enter_context(tc.tile_pool(name="psmall", bufs=1, space="PSUM"))

        # all inputs on the sync queue, smallest first (FIFO DMA engine)
        V16 = pers.tile([128, NB, 2], F8, tag="V16")
        nc.sync.dma_start(out=V16[:], in_=V2[:].rearrange("(kb p) c -> p kb c", p=128))
        tpw = pers.tile([128, NB, K], I16, tag="tpw")
        nc.sync.dma_start(out=tpw[:], in_=tpi[:].rearrange("(m p) k -> p m k", p=128))
        ews16 = pers.tile([128, NB, K], BF16, tag="ews16")
        nc.sync.dma_start(out=ews16[:], in_=ewd[:].rearrange("(m p) k -> p m k", p=128))
        xT16 = pers.tile([128, NB, S], F8, tag="xT16")
        nc.sync.dma_start(out=xT16[:], in_=xT[:].rearrange("(kb p) s -> p kb s", p=128))
        WTh16 = pers.tile([128, NB, H], F8, tag="WTh16")
        nc.sync.dma_start(out=WTh16[:], in_=WTh[:].rearrange("(kb p) s -> p kb s", p=128))
        WTl16 = pers.tile([128, NB, H], F8, tag="WTl16")
        nc.sync.dma_start(out=WTl16[:], in_=WTl[:].rearrange("(kb p) s -> p kb s", p=128))

        # gpsimd: M0 blocks (scatter of ew into dense [s, t])
        M0 = pers.tile([128, NB, S], BF16, tag="M0")
        for m in range(NB):
            nc.gpsimd.local_scatter(M0[:, m, :], ews16[:, m, :], tpw[:, m, :],
                                    channels=128, num_elems=S, num_idxs=K)

        # PE: e_srcT / e_dstT [1, node] = (V col)^T x  (separate 1-row outputs so
        # each lands at base partition 0); V is host-scaled by V_SCALE
        esT = pers.tile([1, S], F32, tag="esT")
        edT = pers.tile([1, S], F32, tag="edT")
        for col, dst in ((1, edT), (0, esT)):
            for n0 in range(0, S, 512):
                pt = psmall.tile([1, 512], F32, tag="ebp")
                for k in range(NB):
                    nc.tensor.matmul(pt[:], V16[:, k, col:col + 1],
                                     xT16[:, k, n0:n0 + 512],
                                     start=(k == 0), stop=(k == NB - 1))
                nc.vector.tensor_scalar(dst[:, n0:n0 + 512], pt[:], 1.0 / V_SCALE, None, op0=OP.mult)

        # PE: broadcast e_dst across partitions (rank-1 matmul with ones)
        ones1r = pers.tile([1, 128], F32, tag="ones1r")
        nc.vector.memset(ones1r[:], 1.0)
        edb = pers.tile([128, S], F32, tag="edb")
        for n0 in range(0, S, 512):
            pt = psmall.tile([128, 512], F32, tag="edbp")
            nc.tensor.matmul(pt[:], ones1r[:], edT[0:1, n0:n0 + 512], start=True, stop=True)
            nc.scalar.copy(out=edb[:, n0:n0 + 512], in_=pt[:])

        # PE: e_src into partitions (transpose via 1-col matmul)
        ones11 = pers.tile([1, 1], F32, tag="ones11")
        nc.vector.memset(ones11[:], 1.0)
        esc = pers.tile([128, NB, 1], F32, tag="esc")
        for m in range(NB):
            pt = psmall.tile([128, 1], F32, tag="escp")
            nc.tensor.matmul(pt[:], esT[0:1, m * 128:(m + 1) * 128], ones11[:],
                             start=True, stop=True)
            nc.vector.tensor_copy(out=esc[:, m, :], in_=pt[:])

        # PE: h [node, feat] bf16, W_SCALE-scaled.  Two fp8 DoubleRow passes
        # (W-hi, then the natural-scale fp8 residual W-lo, which lands in the
        # denormal range) accumulate into ONE psum group — bf16-level weight
        # accuracy at fp8-DR speed, plain copy eviction.
        h16 = pers.tile([128, NB, H], BF16, tag="h16")
        for m in range(NB):
            for n0 in range(0, H, 512):
                pt = psum.tile([128, 512], F32, tag="mmp")
                for i, wt in enumerate((WTh16, WTl16)):
                    for d in range(ND):
                        nc.tensor.matmul(pt[:], xT16[:, 2 * d:2 * d + 2, m * 128:(m + 1) * 128],
                                         wt[:, 2 * d:2 * d + 2, n0:n0 + 512],
                                         start=(i == 0 and d == 0),
                                         stop=(i == 1 and d == ND - 1), perf_mode=DR)
                nc.vector.tensor_copy(out=h16[:, m, n0:n0 + 512], in_=pt[:])

        # Act: all lrelu then all exp (2 act-table loads total); DVE: R = M0 * ez
        zl8 = pers.tile([128, NB, S], BF16, tag="zl8")
        for m in range(NB):
            nc.scalar.activation(zl8[:, m, :], edb[:], AF.Lrelu, bias=esc[:, m, :], alpha=0.2)
        ez8 = pers.tile([128, NB, S], BF16, tag="ez8")
        for m in range(NB):
            nc.scalar.activation(ez8[:, m, :], zl8[:, m, :], AF.Exp)
        R = pers.tile([128, NB, S], BF16, tag="R")
        for m in range(NB):
            nc.vector.tensor_tensor(R[:, m, :], M0[:, m, :], ez8[:, m, :], op=OP.mult)

        # PE: attn^T [1, t] = 1^T R ; arc = 1/(HEADS*W_SCALE) / (attn + 1e-8)
        onesc = pers.tile([128, 1], BF16, tag="onesc")
        nc.vector.memset(onesc[:], 1.0)
        atT = pers.tile([1, S], F32, tag="atT")
        for n0 in range(0, S, 512):
            pt = psmall.tile([1, 512], F32, tag="atp")
            for k in range(NB):
                nc.tensor.matmul(pt[:], onesc[:], R[:, k, n0:n0 + 512],
                                 start=(k == 0), stop=(k == NB - 1))
            nc.vector.tensor_copy(out=atT[:, n0:n0 + 512], in_=pt[:])
        nc.vector.tensor_scalar(atT[:], atT[:], 1e-8, None, op0=OP.add)
        arc = pers.tile([1, S], F32, tag="arc")
        nc.vector.reciprocal(arc[:], atT[:])
        nc.vector.tensor_scalar(arc[:], arc[:], 1.0 / (HEADS * W_SCALE), None, op0=OP.mult)
        rcb = pers.tile([128, S], F32, tag="rcb")
        nc.gpsimd.partition_broadcast(rcb[:], arc[:])

        # PE: out^T [feat, t] = h^T R, scaled by rcb; chunked DMA out
        gsb = pers.tile([128, NB, S], BF16, tag="gsb")
        gTr = gT[:].rearrange("(m p) t -> p m t", p=128)
        for m in range(NB):
            for n0 in range(0, S, 512):
                pt = psum.tile([128, 512], F32, tag="mmp")
                for k in range(NB):
                    nc.tensor.matmul(pt[:], h16[:, k, m * 128:(m + 1) * 128],
                                     R[:, k, n0:n0 + 512],
                                     start=(k == 0), stop=(k == NB - 1))
                nc.vector.tensor_tensor(gsb[:, m, n0:n0 + 512], pt[:], rcb[:, n0:n0 + 512], op=OP.mult)
            nc.sync.dma_start(out=gTr[:, m, :], in_=gsb[:, m, :])
    nc.compile()
    return nc


def _build_D(nc):
    """Attention pool over nodes + 2-layer projection head, one batch per core."""
    x2T = nc.dram_tensor("x2T", [H, S], BF16, kind="ExternalInput")
    x2n = nc.dram_tensor("x2n", [S, H], BF16, kind="ExternalInput")
    wpc = nc.dram_tensor("wpc", [H, 1], BF16, kind="ExternalInput")
    w1T = nc.dram_tensor("w1T", [H, SEM], BF16, kind="ExternalInput")
    b1c = nc.dram_tensor("b1c", [SEM, 1], F32, kind="ExternalInput")
    w2T = nc.dram_tensor("w2T", [SEM, SEM], BF16, kind="ExternalInput")
    b2c = nc.dram_tensor("b2c", [SEM, 1], F32, kind="ExternalInput")
    res = nc.dram_tensor("res", [SEM, 1], F32, kind="ExternalOutput")

    with tile.TileContext(nc) as tc, ExitStack() as ctx:
        pers = ctx.enter_context(tc.tile_pool(name="pers", bufs=1))
        tmp = ctx.enter_context(tc.tile_pool(name="tmp", bufs=3))
        psum = ctx.enter_context(tc.tile_pool(name="psum", bufs=3, space="PSUM"))

        wp16 = pers.tile([128, NB, 1], BF16, tag="wp16")
        nc.sync.dma_start(out=wp16[:], in_=wpc[:].rearrange("(kb p) c -> p kb c", p=128))
        b1f = pers.tile([128, 4, 1], F32, tag="b1f")
        nc.sync.dma_start(out=b1f[:], in_=b1c[:].rearrange("(m p) c -> p m c", p=128))
        b2f = pers.tile([128, 4, 1], F32, tag="b2f")
        nc.sync.dma_start(out=b2f[:], in_=b2c[:].rearrange("(m p) c -> p m c", p=128))
        # x2T column-chunked so psc starts after the first half arrives
        x3T = pers.tile([128, NB, S], BF16, tag="x3T")
        x2Tr = x2T[:].rearrange("(kb p) s -> p kb s", p=128)
        for n0 in range(0, S, 512):
            nc.sync.dma_start(out=x3T[:, :, n0:n0 + 512], in_=x2Tr[:, :, n0:n0 + 512])
        x2t16 = pers.tile([128, NB, H], BF16, tag="x2t16")
        nc.sync.dma_start(out=x2t16[:], in_=x2n[:].rearrange("(tb p) f -> p tb f", p=128))
        w116 = pers.tile([128, NB, SEM], BF16, tag="w116")
        nc.sync.dma_start(out=w116[:], in_=w1T[:].rearrange("(kb p) c -> p kb c", p=128))
        w216 = pers.tile([128, 4, SEM], BF16, tag="w216")
        nc.sync.dma_start(out=w216[:], in_=w2T[:].rearrange("(kb p) c -> p kb c", p=128))

        # preload the Exp act table during the DMAs
        warm = pers.tile([1, 1], F32, tag="warm")
        nc.vector.memset(warm[:], 0.0)
        nc.scalar.activation(warm[:], warm[:], AF.Exp)

        psc = pers.tile([1, S], F32, tag="psc")
        for n0 in range(0, S, 512):
            pt = psum.tile([1, 512], F32, tag="sp")
            for k in range(NB):
                nc.tensor.matmul(pt[:], wp16[:, k, :], x3T[:, k, n0:n0 + 512],
                                 start=(k == 0), stop=(k == NB - 1))
            nc.vector.tensor_copy(out=psc[:, n0:n0 + 512], in_=pt[:])

        mx = pers.tile([1, 1], F32, tag="mx")
        nc.vector.tensor_reduce(mx[:], psc[:], axis=AX.X, op=OP.max)
        nmx = pers.tile([1, 1], F32, tag="nmx")
        nc.vector.tensor_scalar(nmx[:], mx[:], -1.0, None, op0=OP.mult)
        ev = pers.tile([1, S], F32, tag="ev")
        nc.scalar.activation(ev[:], psc[:], AF.Exp, bias=nmx[:])
        sm = pers.tile([1, 1], F32, tag="sm")
        nc.vector.tensor_reduce(sm[:], ev[:], axis=AX.X, op=OP.add)
        rc = pers.tile([1, 1], F32, tag="rc")
        nc.vector.reciprocal(rc[:], sm[:])
        alT = pers.tile([1, S], BF16, tag="alT")
        nc.vector.tensor_scalar(alT[:], ev[:], rc[:], None, op0=OP.mult)

        # transpose alpha into partitions (8 tiny matmuls), then pooled = x2^T @ alpha
        # runs on PE instead of a serial DVE accumulation chain
        ones11 = pers.tile([1, 1], BF16, tag="ones11")
        nc.vector.memset(ones11[:], 1.0)
        alp = pers.tile([128, NB, 1], BF16, tag="alp")
        for tb in range(NB):
            pt = psum.tile([128, 1], F32, tag="sp1")
            nc.tensor.matmul(pt[:], alT[0:1, tb * 128:(tb + 1) * 128], ones11[:],
                             start=True, stop=True)
            nc.vector.tensor_copy(out=alp[:, tb, :], in_=pt[:])
        pld = pers.tile([128, NB, 1], BF16, tag="pld")
        for fb in range(NB):
            pt = psum.tile([128, 1], F32, tag="sp1")
            for tb in range(NB):
                nc.tensor.matmul(pt[:], x2t16[:, tb, fb * 128:(fb + 1) * 128], alp[:, tb, :],
                                 start=(tb == 0), stop=(tb == NB - 1))
            nc.vector.tensor_copy(out=pld[:, fb, :], in_=pt[:])

        hid = pers.tile([128, 4, 1], BF16, tag="hid")
        for m in range(4):
            pt = psum.tile([128, 1], F32, tag="sp1")
            for k in range(NB):
                nc.tensor.matmul(pt[:], w116[:, k, m * 128:(m + 1) * 128], pld[:, k, :],
                                 start=(k == 0), stop=(k == NB - 1))
            nc.scalar.activation(hid[:, m, :], pt[:], AF.Relu, bias=b1f[:, m, :])

        rsb = pers.tile([128, 4, 1], F32, tag="rsb")
        for m in range(4):
            pt = psum.tile([128, 1], F32, tag="sp1")
            for k in range(4):
                nc.tensor.matmul(pt[:], w216[:, k, m * 128:(m + 1) * 128], hid[:, k, :],
                                 start=(k == 0), stop=(k == 3))
            nc.vector.tensor_tensor(rsb[:, m, :], pt[:], b2f[:, m, :], op=OP.add)
        nc.sync.dma_start(out=res[:].rearrange("(m p) c -> p m c", p=128), in_=rsb[:])
    nc.compile()
    return nc


_PROGS = {}


def _get_progs():
    if not _PROGS:
        def mk():
            return bacc.Bacc("TRN2", target_bir_lowering=False, debug=False,
                             enable_asserts=True, num_devices=8)
        _PROGS["A0"] = _build_P0(mk())
        _PROGS["A"] = _build_P1(mk())
        _PROGS["B"] = _build_L(mk())
        _PROGS["C"] = _build_L(mk())
        _PROGS["D"] = _build_D(mk())
    return _PROGS


def kernel(hidden_states, phi_w, psi_w, gat_lin_w, gat_att, wp, w1, b1, w2, b2,
           _profile=None):
    f32 = np.float32
    bf16 = ml_dtypes.bfloat16
    hidden_states = np.asarray(hidden_states, f32)
    progs = _get_progs()
    C = lambda a: np.ascontiguousarray(a)
    times = {}

    def run(tag, in_maps, core_ids):
        r = run_bass_kernel_spmd(progs[tag], in_maps, core_ids=core_ids)
        if _profile is not None:
            times[tag] = r.exec_time_ns
        return r.results

    f8 = ml_dtypes.float8_e4m3
    glw = np.asarray(gat_lin_w, f32)
    ga = np.asarray(gat_att, f32)
    xTb = [C(hidden_states[b].T.astype(bf16)) for b in range(B)]
    xTb8 = [C(hidden_states[b].T.astype(f8)) for b in range(B)]

    # ---- launch P0: M = phi_w.T @ psi_w chunks, V = W^T [a_src|a_dst] ----
    # (reference einsum 'bsd,ed->bse' is x @ phi_w.T, so scores = x M x.T with
    # M = phi_w.T @ psi_w; the contraction runs over the e rows of both.)
    pT = np.asarray(phi_w, f32).astype(bf16)
    sT = C(np.asarray(psi_w, f32).astype(bf16))
    in_0 = []
    for c in range(8):
        l, hd = c // 4, c % 4
        in_0.append({
            "pTc": C(pT[:, c * 128:(c + 1) * 128]),
            "sT": sT,
            "Wn": C((glw[l, hd * H:(hd + 1) * H, :] * W_SCALE).astype(f8)),
            "a2": C((np.stack([ga[l, hd, :H], ga[l, hd, H:]], axis=1) * A_SCALE).astype(f8)),
        })
    r0 = run("A0", in_0, list(range(8)))
    Mfull = C(np.concatenate([r0[c]["Mc"] for c in range(8)], axis=0))
    V2 = [[C((r0[l * 4 + hd]["VT"].T * V_SCALE).astype(f8)) for hd in range(4)] for l in range(2)]

    # ---- launch P1: edge build ----
    in_a = []
    for c in range(8):
        b, rcn = c // 4, c % 4
        in_a.append({
            "xT": xTb[b], "xTc": C(xTb[b][:, rcn * CH:(rcn + 1) * CH]),
            "Mm": Mfull,
            "srcx": C(np.arange(rcn * CH, (rcn + 1) * CH, dtype=np.float32)[:, None]),
        })
    ra = run("A", in_a, list(range(8)))
    topi = np.stack([np.concatenate([ra[b * 4 + r]["topi"] for r in range(4)], 0) for b in range(B)])
    ew = np.stack([np.concatenate([ra[b * 4 + r]["ew"] for r in range(4)], 0) for b in range(B)])
    tpi16 = [C(topi[b].astype(np.int16)) for b in range(B)]
    ew16 = [C(ew[b].astype(bf16)) for b in range(B)]

    # ---- launches P2, P3: the two GAT layers (host pre-sums partials) ----
    xin8 = xTb8
    for li, tag in enumerate(("B", "C")):
        in_l = []
        for c in range(8):
            b, hd = c // 4, c % 4
            w32 = glw[li, hd * H:(hd + 1) * H, :].T * W_SCALE
            wh = w32.astype(f8)
            wl = (w32 - wh.astype(f32)).astype(f8)
            in_l.append({
                "xT": xin8[b],
                "WTh": C(wh), "WTl": C(wl),
                "V2": V2[li][hd],
                "tpi": tpi16[b], "ewd": ew16[b],
            })
        rl = run(tag, in_l, list(range(8)))
        xin8, xacc = [], []
        for b in range(B):
            acc = np.maximum(sum(rl[b * 4 + i]["gT"].astype(f32) for i in range(4)), 0.0)
            xacc.append(acc)
            xin8.append(C(acc.astype(f8)))

    # ---- launch P4: pooling + projection head ----
    in_d = []
    for b in range(B):
        in_d.append({
            "x2T": C(xacc[b].astype(bf16)),
            "x2n": C(xacc[b].T.astype(bf16)),
            "wpc": C(np.asarray(wp, f32).reshape(H, 1).astype(bf16)),
            "w1T": C(np.asarray(w1, f32).T.astype(bf16)),
            "b1c": C(np.asarray(b1, f32)[:, None]),
            "w2T": C(np.asarray(w2, f32).T.astype(bf16)),
            "b2c": C(np.asarray(b2, f32)[:, None]),
        })
    rd = run("D", in_d, [0, 1])
    out = np.stack([rd[b]["res"][:, 0].astype(f32) for b in range(B)])
    if _profile is not None:
        _profile.update(times)
    return out
